# revision 10
# baseline (speedup 1.0000x reference)
"""Trainium2 Bass kernel for nn_CNN_RNN_88347477278730.

Pipeline (data-parallel over batch, 8 rows per core on 8 cores):
  kernel1 (device): chunked fp16 input projection (half hoisted, half
      interleaved into the early recurrence), then the 512-step
      select-policy GRUCell recurrence in full fp16 state with two 4-row
      batch streams for ILP; decisions batched into matmuls + is_gt at
      the end.
  host: compaction (gather kept tokens to the front), new_lens, Ldyn.
  kernel2 (device): compiled per dynamic sequence-length bucket t2
      (multiple of 32 >= max(new_lens)); chunked proj0, 2-layer GRU
      recurrence pipelined with a small lag, per-chunk proj1, Kim-CNN
      convs as shifted matmuls with compile-time pool windows, final
      linear.

All recurrence matmuls are gate-major (lhsT = weight tiles [K=128,
M=128], moving operand = h [K, batch]) so gate tensors land
partition-major where the elementwise engines are fast. The per-step
elementwise chain is 9 ops balanced across Vector/Scalar/GpSimd.
"""

import os
import subprocess
import sys
import tempfile

import numpy as np

# ---------------------------------------------------------------- constants
B, T, E, H, NF = 64, 512, 768, 256, 100
NCORES = 8
BPC = B // NCORES  # batch rows per core
KE = E // 128      # 6 K-tiles over the embedding dim
KH = H // 128      # 2 K-tiles over the hidden dim
GC = (3 * H) // 128  # 6 gate chunks (r: 0-1, z: 2-3, n: 4-5)

_F32 = None  # set lazily to mybir.dt.float32


# ------------------------------------------------------------- tile patch
def _apply_tile_patch():
    """This walrus build rejects >2 sem waits on one SP control instruction;
    split the TileContext tail drain into several drains of <=2 waits."""
    import concourse.tile as tile
    from concourse.vector_clock import ScopedClock, VectorClock

    if getattr(tile.TileContext, "_drain_split_patched", False):
        return

    def _patched(self, tick_clock, wait_clock):
        gc = tick_clock.global_clock
        n = len(gc)
        for start in range(0, n, 1):
            vec = [0] * n
            any_set = False
            for p in range(start, min(start + 1, n)):
                vec[p] = gc[p]
                any_set = any_set or vec[p] > 0
            if not any_set:
                continue
            d = self.nc.sync.drain()
            wait_clock.add_sem_waits(d.ins, ScopedClock({None: VectorClock(vec)}))
        self.nc.all_engine_barrier()
        assert self.sems is not None
        popped = self.nc._tile_sem_poison_stack.pop()
        assert popped is self._sem_poison
        self.nc.clear_and_free_semaphores(list(self.sems.allocated().values()))
        self.nc.all_engine_barrier()

    tile.TileContext._drain_and_barrier = _patched
    tile.TileContext._drain_split_patched = True


# ------------------------------------------------------------- gumbel (CPU)
def _gumbel_cpu():
    """jax.random.gumbel(key(42), (T-1, B, 2), f32) — computed in a CPU-jax
    subprocess so the accelerator backend is never involved (it must be
    bit-identical to the reference's CPU computation)."""
    path = os.path.join(tempfile.mkdtemp(), "gumbel.npy")
    code = (
        "import numpy as np, jax, jax.numpy as jnp\n"
        f"g = jax.random.gumbel(jax.random.key(42), ({T - 1}, {B}, 2), jnp.float32)\n"
        f"np.save({path!r}, np.asarray(g))\n"
    )
    env = dict(os.environ)
    env["TRN_TERMINAL_POOL_IPS"] = ""
    env["JAX_PLATFORMS"] = "cpu"
    extra = [p for p in sys.path if p and os.path.isdir(p)]
    env["PYTHONPATH"] = os.pathsep.join(extra)
    subprocess.run([sys.executable, "-c", code], env=env, check=True, capture_output=True)
    return np.load(path)


# ------------------------------------------------------------- host packing
def _pack_T(a2d):
    """[rows(=128*k), cols] -> [128, k, cols] weight-tile layout."""
    rows, cols = a2d.shape
    k = rows // 128
    return np.ascontiguousarray(a2d.reshape(k, 128, cols).transpose(1, 0, 2)).astype(np.float32)


def _pack_bias(b1d):
    """[128*k] -> [128, k]"""
    k = b1d.shape[0] // 128
    return np.ascontiguousarray(b1d.reshape(k, 128).T).astype(np.float32)


def _pack_embT(emb_rows, t_len=T):
    """[bpc, T, E] -> [KE, 128, bpc*T] (e-major tiles, free dims (b, t))."""
    bpc = emb_rows.shape[0]
    x = emb_rows.transpose(2, 0, 1).reshape(KE, 128, bpc * t_len)
    return np.ascontiguousarray(x).astype(np.float32)


def _pack_gru_weights(Wih, Whh, bih, bhh):
    """Returns (wihT, whhT, bias_proj, bhhn_rep) packings.

    bias_proj folds bih+bhh for the r,z chunks (added once at projection
    time); n chunks get bih only, with bhh_n applied per-step (it must be
    added to h@Whh_n *before* the r* multiply)."""
    wihT = _pack_T(np.ascontiguousarray(Wih.T))  # [128, KE or KH, 3H]
    whhT = _pack_T(np.ascontiguousarray(Whh.T))  # [128, KH, 3H]
    bias = np.empty(3 * H, np.float32)
    bias[: 2 * H] = bih[: 2 * H] + bhh[: 2 * H]
    bias[2 * H:] = bih[2 * H:]
    bias_proj = _pack_bias(bias)  # [128, GC]
    bhhn = _pack_bias(bhh[2 * H:])  # [128, KH]
    bhhn_rep = np.ascontiguousarray(
        np.broadcast_to(bhhn[:, :, None], (128, KH, BPC))
    ).astype(np.float32)
    return wihT, whhT, bias_proj, bhhn_rep


def _np_f16():
    from concourse import mybir

    return mybir.dt.np(mybir.dt.float16)


# ------------------------------------------------------------- bass builders
def _mk_nc():
    import concourse.bass as bass

    return bass.Bass("TRN2", target_bir_lowering=False, debug=False, num_devices=1)


def _split_excess_waits(nc, max_waits=1):
    """This walrus build can only encode ~2 sem waits per instruction
    (setupSyncWait 'Too many sync wait commands'). Hoist excess waits onto
    same-engine NoOps inserted just before the over-subscribed instruction;
    engine queues execute in order, so the wait semantics are identical."""
    from concourse import mybir

    nid = [0]
    for f in nc.m.functions:
        for bb in f.blocks:
            out = []
            changed = False
            for inst in bb.instructions:
                si = inst.sync_info
                lim = max_waits
                if si is not None and si.on_wait and len(si.on_wait) > lim:
                    waits = list(si.on_wait)
                    extra, keep = waits[:-lim], waits[-lim:]
                    for j in range(0, len(extra), max_waits):
                        nop = mybir.InstNoOp(
                            name=f"I-waitnop-{nid[0]}", ins=[], outs=[])
                        nid[0] += 1
                        nop.engine = inst.engine
                        nop.sync_info = mybir.SyncInfo(
                            on_wait=extra[j: j + max_waits], on_update=[])
                        nc.register_instruction(nop, overwrite=True)
                        out.append(nop)
                    inst.sync_info = mybir.SyncInfo(
                        on_wait=keep, on_update=list(si.on_update or []))
                    changed = True
                out.append(inst)
            if changed:
                bb.instructions = out
    return nc


def _proj_builder(nc, tc, misc_pool, dma_pool, big_ps, src_dram, wihT, biasc,
                  biasc_rep, gi, t_len, n_k, act, alu, f16, f32, tag):
    """Returns (emit_prefix, groups): chunked input projection.

    emit_prefix() emits the first t-half; `groups` is a list of closures,
    each emitting one (b, c) group of the second t-half (to be interleaved
    into the early recurrence steps). Copies alternate ACT (per-partition
    bias) / DVE (bias_rep tensor) to balance engines."""
    HALF = t_len // 2
    src_tiles = {}

    def _src(hb, b):
        key = (hb, b)
        if key not in src_tiles:
            s = dma_pool.tile([128, n_k, HALF], f16, tag=f"{tag}src")
            for k in range(n_k):
                nc.sync.dma_start(
                    s[:, k, :],
                    src_dram[k, :, b * t_len + hb * HALF: b * t_len + hb * HALF + HALF],
                )
            src_tiles[key] = s
        return src_tiles[key]

    def _group(hb, b, c):
        src = _src(hb, b)
        ps = big_ps.tile([128, HALF], f32, tag="bps")
        for k in range(n_k):
            nc.tensor.matmul(
                ps[:], wihT[:, k, c * 128: (c + 1) * 128], src[:, k, :],
                start=(k == 0), stop=(k == n_k - 1),
            )
        t0 = hb * HALF
        dst = gi[:, c, b, t0: t0 + HALF]
        if (b * GC + c) % 2 == 0:
            nc.scalar.activation(dst, ps[:], act.Identity, bias=biasc[:, c: c + 1])
        else:
            nc.vector.tensor_tensor(dst, ps[:], biasc_rep[:, c, :], alu.add)

    def emit_prefix():
        for b in range(BPC):
            for c in range(GC):
                _group(0, b, c)

    groups = [
        (lambda b=b, c=c: _group(1, b, c))
        for b in range(BPC) for c in range(GC)
    ]
    return emit_prefix, groups


def build_kernel1(t_len=T):
    """Select-policy kernel: fp16 everywhere; ONE merged 8-row stream (the
    per-step serial chain latency is the period — extra streams only add
    engine-queue coupling); gi_rz and bhh_n preloaded into PSUM off-chain so
    the gate matmuls accumulate straight onto them (start=False after a
    has_written-priming dummy matmul); 7-op chain; decisions batched at
    the end."""
    import concourse.tile as tile
    from concourse import mybir

    _apply_tile_patch()
    nc = _mk_nc()
    f32 = mybir.dt.float32
    f16 = mybir.dt.float16
    act = mybir.ActivationFunctionType
    alu = mybir.AluOpType
    HALF = t_len // 2

    embT_d = nc.dram_tensor("embT", [KE, 128, BPC * t_len], f16, kind="ExternalInput").ap()
    wihcT_d = nc.dram_tensor("wihcT", [128, KE, 3 * H], f16, kind="ExternalInput").ap()
    biasc_d = nc.dram_tensor("biasc", [128, GC], f32, kind="ExternalInput").ap()
    biascr_d = nc.dram_tensor("biascr", [128, GC, HALF], f32, kind="ExternalInput").ap()
    whh16_d = nc.dram_tensor("whh16", [128, KH, 3 * H], f16, kind="ExternalInput").ap()
    bhhnc_d = nc.dram_tensor("bhhnc", [128, KH, BPC], f32, kind="ExternalInput").ap()
    wdiffT_d = nc.dram_tensor("wdiffT", [128, KH, 1], f16, kind="ExternalInput").ap()
    ncdiff_d = nc.dram_tensor("ncdiff", [1, BPC * t_len], f32, kind="ExternalInput").ap()
    ks_d = nc.dram_tensor("ks", [1, BPC * t_len], f32, kind="ExternalOutput").ap()

    with tile.TileContext(nc) as tc:
        from contextlib import ExitStack

        with ExitStack() as ctx:
            wpool = ctx.enter_context(tc.tile_pool(name="weights", bufs=1))
            gipool = ctx.enter_context(tc.tile_pool(name="gi", bufs=1))
            hpool = ctx.enter_context(tc.tile_pool(name="hist", bufs=1))
            dma_pool = ctx.enter_context(tc.tile_pool(name="dma", bufs=2))
            big_ps = ctx.enter_context(tc.tile_pool(name="bigps", bufs=2, space="PSUM"))
            rz_psp = ctx.enter_context(tc.tile_pool(name="rzps", bufs=1, space="PSUM"))
            n_psp = ctx.enter_context(tc.tile_pool(name="nps", bufs=1, space="PSUM"))
            sb_pool = ctx.enter_context(tc.tile_pool(name="gates", bufs=2))
            misc = ctx.enter_context(tc.tile_pool(name="misc", bufs=1))

            def _load(pool, dram, shape, tag, dt=f32):
                t_ = pool.tile(shape, dt, tag=tag)
                nc.sync.dma_start(t_[:], dram[:])
                return t_

            wihcT = _load(wpool, wihcT_d, [128, KE, 3 * H], "wihcT", f16)
            biasc = _load(wpool, biasc_d, [128, GC], "biasc")
            biascr = _load(wpool, biascr_d, [128, GC, HALF], "biascr")
            whh16 = _load(wpool, whh16_d, [128, KH, 3 * H], "whh16", f16)
            bhhnc_rep = _load(wpool, bhhnc_d, [128, KH, BPC], "bhhnc")
            wdiffT = _load(misc, wdiffT_d, [128, KH, 1], "wdiffT", f16)
            ncdiff = _load(misc, ncdiff_d, [1, BPC * t_len], "ncdiff")
            zs = misc.tile([1, 128], f16, tag="zs")
            zx = misc.tile([1, 4 * BPC], f16, tag="zx")
            nc.vector.memset(zs[:], 0.0)
            nc.vector.memset(zx[:], 0.0)

            # gi layout: [128, GC, BPC, t] (t innermost: contiguous proj copies)
            gi = gipool.tile([128, GC, BPC, t_len], f16, tag="gi")
            emit_prefix, groups = _proj_builder(
                nc, tc, misc, dma_pool, big_ps, embT_d, wihcT, biasc, biascr,
                gi, t_len, KE, act, alu, f16, f32, "proj")
            emit_prefix()

            hist = hpool.tile([128, KH, BPC, t_len], f16, tag="hist")
            rz_ps = rz_psp.tile([128, 4, BPC], f32, tag="rz")
            n_ps = n_psp.tile([128, KH, BPC], f32, tag="n")

            # prime has_written for the preload+accumulate banks
            nc.tensor.matmul(rz_ps[:], zs[:], zx[:], start=True, stop=True)
            nc.tensor.matmul(n_ps[:], zs[:], zx[0:1, 0: KH * BPC], start=True, stop=True)
            # initial preloads for t=0
            nc.vector.tensor_copy(rz_ps[:], gi[:, 0:4, :, 0])
            nc.scalar.activation(n_ps[:], bhhnc_rep[:], act.Identity)

            def _gh(w, rhs, chunks, last=False):
                """Accumulate Whh @ rhs into the gate banks (start=False)."""
                for c in chunks:
                    dst = rz_ps[:, c, :] if c < 4 else n_ps[:, c - 4, :]
                    for k in range(KH):
                        nc.tensor.matmul(
                            dst, w[:, k, c * 128: (c + 1) * 128], rhs[:, k, :],
                            start=False, stop=(last and c == chunks[-1] and k == KH - 1),
                            skip_group_check=True,
                        )

            def emit_step(t):
                # gh(t) was already accumulated by step t-1 (MM-split: the
                # Whh@nn part right after tanh, the Whh@d part after op8).
                # Chain: sigma_r -> x r -> +gi_n -> tanh -> (h_prev-nn) ->
                # x z -> MM_d; sigma_z, preloads, MM_nn, h-write off-chain.
                rz = sb_pool.tile([128, 4, BPC], f32, tag="rz")
                tmp = sb_pool.tile([128, KH, BPC], f32, tag="tmp")
                nc.scalar.activation(rz[:, 0:2, :], rz_ps[:, 0:2, :], act.Sigmoid)
                nc.scalar.activation(rz[:, 2:4, :], rz_ps[:, 2:4, :], act.Sigmoid)
                nc.vector.tensor_tensor(tmp[:], n_ps[:], rz[:, 0:KH, :], alu.mult)
                nc.vector.tensor_tensor(tmp[:], tmp[:], gi[:, 4:GC, :, t], alu.add)
                if t + 1 < t_len:
                    nc.vector.tensor_copy(rz_ps[:], gi[:, 0:4, :, t + 1])
                nn16 = sb_pool.tile([128, KH, BPC], f16, tag="nn")
                nc.scalar.activation(nn16[:], tmp[:], act.Tanh)
                if t + 1 < t_len:
                    nc.scalar.activation(n_ps[:], bhhnc_rep[:], act.Identity)
                    _gh(whh16, nn16, (0, 1))
                d = sb_pool.tile([128, KH, BPC], f32, tag="dd")
                if t > 0:
                    nc.vector.tensor_tensor(d[:], hist[:, :, :, t - 1], nn16[:], alu.subtract)
                else:
                    nc.vector.tensor_scalar(d[:], nn16[:], -1.0, None, alu.mult)
                d16 = sb_pool.tile([128, KH, BPC], f16, tag="dd16")
                nc.vector.tensor_tensor(d16[:], rz[:, 2:4, :], d[:], alu.mult)
                if t + 1 < t_len:
                    # critical-first PE order: the r-chunk d-MMs gate the next
                    # step's sigma_r, so they go right behind the r-chunk
                    # nn-MMs; everything else drains during the next chain.
                    _gh(whh16, d16, (0, 1))
                    _gh(whh16, nn16, (2, 3))
                    _gh(whh16, d16, (2, 3))
                    _gh(whh16, nn16, (4, 5))
                    _gh(whh16, d16, (4, 5), last=True)
                nc.gpsimd.tensor_tensor(hist[:, :, :, t], nn16[:], d16[:], alu.add)

            gidx = 0
            for t in range(t_len):
                emit_step(t)
                if t % 2 == 1 and gidx < len(groups):
                    groups[gidx]()
                    gidx += 1
            while gidx < len(groups):
                groups[gidx]()
                gidx += 1

            # ---- batched decisions: ks[b,t] = (h_t . wdiff > ncdiff) ----
            ks_sb = misc.tile([1, BPC * t_len], f32, tag="kssb")
            for b in range(BPC):
                dps = big_ps.tile([1, t_len], f32, tag="bps")
                for k in range(KH):
                    nc.tensor.matmul(
                        dps[:], wdiffT[:, k, :], hist[:, k, b, :],
                        start=(k == 0), stop=(k == KH - 1),
                    )
                nc.vector.tensor_tensor(
                    ks_sb[0:1, b * t_len: (b + 1) * t_len], dps[:],
                    ncdiff[0:1, b * t_len: (b + 1) * t_len], alu.is_gt,
                )
            nc.sync.dma_start(ks_d[:], ks_sb[:])

    return _split_excess_waits(nc)


def build_kernel2(t2, kf3, kf4, kf5):
    """GRU0/GRU1 + convs + pooling + final linear at dynamic length t2.

    Both layers use the k1-style low-latency step (PSUM preloads, split
    sigma, DVE-resident chain); no matmul split (two chains share the PE,
    so the per-wave PE budget matters more than each chain's MM segment).
    proj1 computed per-D-chunk from the o1 history into a gi1 buffer;
    conv max-pool windows (kf*) are compile-time constants."""
    import concourse.tile as tile
    from concourse import mybir

    _apply_tile_patch()
    nc = _mk_nc()
    f32 = mybir.dt.float32
    f16 = mybir.dt.float16
    act = mybir.ActivationFunctionType
    alu = mybir.AluOpType
    D = 32
    LAG = D + 8
    HALF = t2 // 2

    nembT_d = nc.dram_tensor("nembT", [KE, 128, BPC * t2], f16, kind="ExternalInput").ap()
    wih0T_d = nc.dram_tensor("wih0T", [128, KE, 3 * H], f16, kind="ExternalInput").ap()
    whh0T_d = nc.dram_tensor("whh0T", [128, KH, 3 * H], f16, kind="ExternalInput").ap()
    bias0_d = nc.dram_tensor("bias0", [128, GC], f32, kind="ExternalInput").ap()
    bias0r_d = nc.dram_tensor("bias0r", [128, GC, HALF], f32, kind="ExternalInput").ap()
    bhhn0_d = nc.dram_tensor("bhhn0", [128, KH, BPC], f32, kind="ExternalInput").ap()
    wih1T_d = nc.dram_tensor("wih1T", [128, KH, 3 * H], f16, kind="ExternalInput").ap()
    whh1T_d = nc.dram_tensor("whh1T", [128, KH, 3 * H], f16, kind="ExternalInput").ap()
    bias1_d = nc.dram_tensor("bias1", [128, GC], f32, kind="ExternalInput").ap()
    bias1r_d = nc.dram_tensor("bias1r", [128, GC, D], f32, kind="ExternalInput").ap()
    bhhn1_d = nc.dram_tensor("bhhn1", [128, KH, BPC], f32, kind="ExternalInput").ap()
    vt_d = nc.dram_tensor("vt", [1, BPC * t2], f16, kind="ExternalInput").ap()
    cw_d = nc.dram_tensor("cw", [128, 12, KH, NF], f16, kind="ExternalInput").ap()
    cb_d = nc.dram_tensor("cb", [NF, 3], f32, kind="ExternalInput").ap()
    woutT_d = nc.dram_tensor("woutT", [NF, 3], f32, kind="ExternalInput").ap()
    bout_d = nc.dram_tensor("bout", [1, 1], f32, kind="ExternalInput").ap()
    out_d = nc.dram_tensor("out", [1, BPC], f32, kind="ExternalOutput").ap()

    FS = (3, 4, 5)
    KFS = (kf3, kf4, kf5)

    with tile.TileContext(nc) as tc:
        from contextlib import ExitStack

        with ExitStack() as ctx:
            wpool = ctx.enter_context(tc.tile_pool(name="weights", bufs=1))
            gipool = ctx.enter_context(tc.tile_pool(name="gi", bufs=1))
            opool = ctx.enter_context(tc.tile_pool(name="obuf", bufs=1))
            dma_pool = ctx.enter_context(tc.tile_pool(name="dma", bufs=2))
            big_ps = ctx.enter_context(tc.tile_pool(name="bigps", bufs=2, space="PSUM"))
            rz0_psp = ctx.enter_context(tc.tile_pool(name="rz0ps", bufs=1, space="PSUM"))
            n0_psp = ctx.enter_context(tc.tile_pool(name="n0ps", bufs=1, space="PSUM"))
            rz1_psp = ctx.enter_context(tc.tile_pool(name="rz1ps", bufs=1, space="PSUM"))
            n1_psp = ctx.enter_context(tc.tile_pool(name="n1ps", bufs=1, space="PSUM"))
            sb_pool = ctx.enter_context(tc.tile_pool(name="gates", bufs=2))
            misc = ctx.enter_context(tc.tile_pool(name="misc", bufs=1))

            def _load(pool, dram, shape, tag, dt=f32):
                t_ = pool.tile(shape, dt, tag=tag)
                nc.sync.dma_start(t_[:], dram[:])
                return t_

            wih0T = _load(wpool, wih0T_d, [128, KE, 3 * H], "wih0", f16)
            whh0T = _load(wpool, whh0T_d, [128, KH, 3 * H], "whh0", f16)
            bias0 = _load(wpool, bias0_d, [128, GC], "bias0")
            bias0r = _load(wpool, bias0r_d, [128, GC, HALF], "bias0r")
            bhhn0 = _load(wpool, bhhn0_d, [128, KH, BPC], "bhhn0")
            wih1T = _load(wpool, wih1T_d, [128, KH, 3 * H], "wih1", f16)
            whh1T = _load(wpool, whh1T_d, [128, KH, 3 * H], "whh1", f16)
            bias1 = _load(wpool, bias1_d, [128, GC], "bias1")
            bias1r = _load(wpool, bias1r_d, [128, GC, D], "bias1r")
            bhhn1 = _load(wpool, bhhn1_d, [128, KH, BPC], "bhhn1")
            cw = _load(wpool, cw_d, [128, 12, KH, NF], "cw", f16)
            cb = _load(misc, cb_d, [NF, 3], "cb")
            woutT = _load(misc, woutT_d, [NF, 3], "woutT")
            bout = _load(misc, bout_d, [1, 1], "bout")
            vt = _load(misc, vt_d, [1, BPC * t2], "vt", f16)
            zs = misc.tile([1, 128], f16, tag="zs")
            zx = misc.tile([1, 4 * BPC], f16, tag="zx")
            nc.vector.memset(zs[:], 0.0)
            nc.vector.memset(zx[:], 0.0)

            gi0 = gipool.tile([128, GC, BPC, t2], f16, tag="gi0")
            gi1 = gipool.tile([128, GC, BPC, t2], f16, tag="gi1")
            o1 = opool.tile([128, KH, BPC, t2], f16, tag="o1")
            o2 = opool.tile([128, KH, BPC, t2], f16, tag="o2")

            emit_prefix, groups = _proj_builder(
                nc, tc, misc, dma_pool, big_ps, nembT_d, wih0T, bias0, bias0r,
                gi0, t2, KE, act, alu, f16, f32, "proj0")
            emit_prefix()

            rz0 = rz0_psp.tile([128, 4, BPC], f32, tag="rz0")
            n0 = n0_psp.tile([128, KH, BPC], f32, tag="n0")
            rz1 = rz1_psp.tile([128, 4, BPC], f32, tag="rz1")
            n1 = n1_psp.tile([128, KH, BPC], f32, tag="n1")

            for ps_t in (rz0, rz1):
                nc.tensor.matmul(ps_t[:], zs[:], zx[:], start=True, stop=True)
            # initial preloads (rz banks only; n banks use start=True matmuls)
            nc.vector.tensor_copy(rz0[:], gi0[:, 0:4, :, 0])

            def emit_step(t, gi, whh, bhhn, hist, prev, rz_ps, n_ps, sfx, tl):
                """One GRU layer step; hist[t] <- GRU(hist[t-1], gi[t]).
                rz bank: preload+accumulate; n bank: plain start=True."""
                if t > 0:
                    h_prev = hist[:, :, :, t - 1]
                    for c in (0, 1, 2, 3, 4, 5):
                        dst = rz_ps[:, c, :] if c < 4 else n_ps[:, c - 4, :]
                        for k in range(KH):
                            nc.tensor.matmul(
                                dst, whh[:, k, c * 128: (c + 1) * 128],
                                h_prev[:, k, :],
                                start=(c >= 4 and k == 0), stop=(k == KH - 1),
                                skip_group_check=True,
                            )
                rz = sb_pool.tile([128, 4, BPC], f32, tag="rz" + sfx)
                tmp = sb_pool.tile([128, KH, BPC], f32, tag="tmp" + sfx)
                nc.scalar.activation(rz[:], rz_ps[:], act.Sigmoid)
                if t > 0:
                    nc.vector.tensor_tensor(tmp[:], n_ps[:], bhhn[:], alu.add)
                else:
                    nc.vector.tensor_copy(tmp[:], bhhn[:])
                nc.vector.tensor_tensor(tmp[:], tmp[:], rz[:, 0:KH, :], alu.mult)
                if t + 1 < tl:
                    nc.vector.tensor_copy(rz_ps[:], gi[:, 0:4, :, t + 1])
                nc.gpsimd.tensor_tensor(tmp[:], tmp[:], gi[:, 4:GC, :, t], alu.add)
                nn16 = sb_pool.tile([128, KH, BPC], f16, tag="nn" + sfx)
                nc.scalar.activation(nn16[:], tmp[:], act.Tanh)
                d = sb_pool.tile([128, KH, BPC], f32, tag="dd" + sfx)
                if t > 0:
                    nc.gpsimd.tensor_tensor(d[:], hist[:, :, :, t - 1], nn16[:], alu.subtract)
                else:
                    nc.gpsimd.tensor_scalar(d[:], nn16[:], -1.0, None, alu.mult)
                d16 = sb_pool.tile([128, KH, BPC], f16, tag="d6" + sfx)
                nc.vector.tensor_tensor(d16[:], rz[:, 2:4, :], d[:], alu.mult)
                nc.gpsimd.tensor_tensor(hist[:, :, :, t], nn16[:], d16[:], alu.add)

            def emit_proj1_chunk(ci):
                t0, t1 = ci * D, (ci + 1) * D
                for b in range(BPC):
                    for c in range(GC):
                        ps = big_ps.tile([128, D], f32, tag="bps")
                        for k in range(KH):
                            nc.tensor.matmul(
                                ps[:], wih1T[:, k, c * 128: (c + 1) * 128],
                                o1[:, k, b, t0:t1],
                                start=(k == 0), stop=(k == KH - 1),
                            )
                        dst = gi1[:, c, b, t0:t1]
                        if (b * GC + c) % 2 == 0:
                            nc.scalar.activation(
                                dst, ps[:], act.Identity, bias=bias1[:, c: c + 1])
                        else:
                            nc.vector.tensor_tensor(dst, ps[:], bias1r[:, c, :], alu.add)

            l1_started = [False]
            gidx = 0
            for w in range(t2 + LAG):
                if w < t2:
                    emit_step(w, gi0, whh0T, bhhn0, o1, o1, rz0, n0, "0", t2)
                    if w % 2 == 1 and gidx < len(groups):
                        groups[gidx]()
                        gidx += 1
                if w >= LAG:
                    t = w - LAG
                    if not l1_started[0]:
                        nc.vector.tensor_copy(rz1[:], gi1[:, 0:4, :, 0])
                        l1_started[0] = True
                    emit_step(t, gi1, whh1T, bhhn1, o2, o2, rz1, n1, "1", t2)
                if w < t2 and w % D == D - 1:
                    emit_proj1_chunk(w // D)
            while gidx < len(groups):
                groups[gidx]()
                gidx += 1

            # ---- zero o2 past new_lens: o2 *= vt ----
            # partition-broadcast vt via a K=1 ones-matmul (PE outer product)
            ones_sb = misc.tile([1, 128], f16, tag="ones")
            nc.vector.memset(ones_sb[:], 1.0)
            for b in range(BPC):
                vtb = big_ps.tile([128, t2], f32, tag="bps")
                nc.tensor.matmul(
                    vtb[:], ones_sb[:], vt[0:1, b * t2: (b + 1) * t2],
                    start=True, stop=True,
                )
                for k in range(KH):
                    nc.vector.tensor_tensor(
                        o2[:, k, b, :], o2[:, k, b, :], vtb[:], alu.mult
                    )

            # ---- convs + relu + max-pool over compile-time window ----
            pooled = misc.tile([NF, 3, BPC], f32, tag="pooled")
            for b in range(BPC):
                for fi, fs in enumerate(FS):
                    nw = t2 - fs + 1
                    kf = KFS[fi]
                    ps = big_ps.tile([NF, t2], f32, tag="bps")
                    m0 = sum(FS[:fi])  # flat (fs,dt) base index
                    first = True
                    for dt_ in range(fs):
                        for k in range(KH):
                            nc.tensor.matmul(
                                ps[:, :nw],
                                cw[:, m0 + dt_, k, :],
                                o2[:, k, b, dt_: dt_ + nw],
                                start=first,
                                stop=(dt_ == fs - 1 and k == KH - 1),
                            )
                            first = False
                    crelu = sb_pool.tile([NF, t2], f32, tag="crelu")
                    nc.scalar.activation(
                        crelu[:, :kf], ps[:, :kf], act.Relu, bias=cb[:, fi: fi + 1]
                    )
                    nc.vector.tensor_reduce(
                        pooled[:, fi, b: b + 1], crelu[:, :kf],
                        mybir.AxisListType.X, alu.max,
                    )

            # ---- final linear ----
            fps = big_ps.tile([1, BPC], f32, tag="bps")
            for fi in range(3):
                nc.tensor.matmul(
                    fps[:], woutT[:, fi: fi + 1], pooled[:, fi, :],
                    start=(fi == 0), stop=(fi == 2),
                )
            out_sb = misc.tile([1, BPC], f32, tag="outsb")
            nc.scalar.activation(out_sb[:], fps[:], act.Identity, bias=bout[0:1, 0:1])
            nc.sync.dma_start(out_d[:], out_sb[:])

    return _split_excess_waits(nc)


# ------------------------------------------------------------- host orchestration
def _host_pack_k1(inputs, gumbel, t_len=T):
    f16 = _np_f16()
    emb = np.asarray(inputs["embedded"], np.float32)
    mask = np.asarray(inputs["mask"])
    lens = mask.sum(1)
    maxlen = int(lens.max())

    wihcT, whhT, biasc, bhhnc = _pack_gru_weights(
        inputs["Wih_c"], inputs["Whh_c"], inputs["bih_c"], inputs["bhh_c"])
    wdiff = (inputs["Wsel"][1] - inputs["Wsel"][0]).astype(np.float32)
    wdiffT = np.ascontiguousarray(wdiff.reshape(KH, 128).T[:, :, None])
    bdiff = float(inputs["bsel"][1] - inputs["bsel"][0])

    # ncdiff[b, t]: k_t = (h.wdiff > ncdiff); forced off when t >= maxlen-1
    ncdiff = np.full((B, t_len), 1.0e30, np.float32)
    upto = min(maxlen - 1, t_len)
    for t in range(1, upto):
        ncdiff[:, t] = -(bdiff + gumbel[t - 1, :, 1] - gumbel[t - 1, :, 0])

    biascr = np.ascontiguousarray(
        np.broadcast_to(biasc[:, :, None], (128, GC, t_len // 2))).astype(np.float32)

    in_maps = []
    for c in range(NCORES):
        rows = slice(c * BPC, (c + 1) * BPC)
        in_maps.append({
            "embT": _pack_embT(emb[rows, :t_len], t_len).astype(f16),
            "wihcT": wihcT.astype(f16),
            "biasc": biasc,
            "biascr": biascr,
            "whh16": whhT.astype(f16),
            "bhhnc": bhhnc,
            "wdiffT": wdiffT.astype(f16),
            "ncdiff": np.ascontiguousarray(
                ncdiff[rows].reshape(1, BPC * t_len)),
        })
    return in_maps, lens, maxlen


def _host_compact(inputs, ks_full, lens, maxlen, t_len=T):
    """ks_full: [B, t_len] decision bits (row t=0 ignored; selected[:,0]=1)."""
    emb = np.asarray(inputs["embedded"], np.float32)
    selected = np.zeros((B, t_len), np.int64)
    selected[:, 0] = 1
    selected[:, 1:] = ks_full[:, 1:]
    pos = np.arange(t_len)
    sel_valid = np.where(pos[None, :] < (lens - 1)[:, None], selected, 0)
    new_mask = np.where(pos[None, :] == (lens - 1)[:, None], 1, sel_valid)
    new_lens = new_mask.sum(1)
    Ldyn = max(int(new_lens.max()), 7)

    t2 = max(-(-Ldyn // 64) * 64, 64)
    new_emb = np.zeros((B, t2, E), np.float32)
    for b in range(B):
        idx = np.nonzero(new_mask[b])[0]
        new_emb[b, : len(idx)] = emb[b, idx]
    return new_emb, new_lens, Ldyn, t2


def _host_pack_k2(inputs, new_emb, new_lens, Ldyn, t2):
    f16 = _np_f16()
    wih0T, whh0T, bias0, bhhn0 = _pack_gru_weights(
        inputs["Wih0"], inputs["Whh0"], inputs["bih0"], inputs["bhh0"])
    wih1T, whh1T, bias1, bhhn1 = _pack_gru_weights(
        inputs["Wih1"], inputs["Whh1"], inputs["bih1"], inputs["bhh1"])

    FS = (3, 4, 5)
    cw = np.zeros((128, 12, KH, NF), np.float32)
    cb = np.zeros((NF, 3), np.float32)
    m = 0
    for fi, fs in enumerate(FS):
        w = np.asarray(inputs[f"conv_w{fs}"], np.float32)  # [NF,1,fs,H]
        cb[:, fi] = np.asarray(inputs[f"conv_b{fs}"], np.float32)
        for dt_ in range(fs):
            wt = w[:, 0, dt_, :].T  # [H, NF]
            cw[:, m, :, :] = wt.reshape(KH, 128, NF).transpose(1, 0, 2)
            m += 1

    woutT = np.ascontiguousarray(
        np.asarray(inputs["Wout"], np.float32)[0].reshape(3, NF).T)
    bout = np.asarray(inputs["bout"], np.float32).reshape(1, 1)

    vt_full = (np.arange(t2)[None, :] < new_lens[:, None]).astype(np.float32)

    bias0r = np.ascontiguousarray(
        np.broadcast_to(bias0[:, :, None], (128, GC, t2 // 2))).astype(np.float32)
    bias1r = np.ascontiguousarray(
        np.broadcast_to(bias1[:, :, None], (128, GC, 32))).astype(np.float32)

    in_maps = []
    for c in range(NCORES):
        rows = slice(c * BPC, (c + 1) * BPC)
        in_maps.append({
            "nembT": _pack_embT(new_emb[rows], t2).astype(f16),
            "wih0T": wih0T.astype(f16), "whh0T": whh0T.astype(f16),
            "bias0": bias0, "bias0r": bias0r, "bhhn0": bhhn0,
            "wih1T": wih1T.astype(f16), "whh1T": whh1T.astype(f16),
            "bias1": bias1, "bias1r": bias1r, "bhhn1": bhhn1,
            "vt": np.ascontiguousarray(
                vt_full[rows].reshape(1, BPC * t2)).astype(f16),
            "cw": cw.astype(f16), "cb": cb,
            "woutT": woutT, "bout": bout,
        })
    return in_maps


_NC_CACHE = {}


def _get_nc1(t_len=T):
    key = (1, t_len)
    if key not in _NC_CACHE:
        _NC_CACHE[key] = build_kernel1(t_len)
    return _NC_CACHE[key]


def _get_nc2(t2, kfs):
    key = (2, t2, kfs)
    if key not in _NC_CACHE:
        _NC_CACHE[key] = build_kernel2(t2, *kfs)
    return _NC_CACHE[key]


TRACE = False  # set True (with an NTFF hook registered) to collect exec times
LAST_STATS = {}


def kernel(**inputs):
    from concourse import bass_utils

    gumbel = _gumbel_cpu()
    core_ids = list(range(NCORES))

    in_maps1, lens, maxlen = _host_pack_k1(inputs, gumbel)
    nc1 = _get_nc1()
    res1 = bass_utils.run_bass_kernel_spmd(nc1, in_maps1, core_ids, trace=TRACE)
    ks_full = np.concatenate(
        [res1.results[c]["ks"].reshape(BPC, T) for c in range(NCORES)], axis=0)

    new_emb, new_lens, Ldyn, t2 = _host_compact(inputs, ks_full, lens, maxlen)
    kfs = tuple(min(Ldyn - fs + 1, t2 - fs + 1) for fs in (3, 4, 5))
    in_maps2 = _host_pack_k2(inputs, new_emb, new_lens, Ldyn, t2)
    nc2 = _get_nc2(t2, kfs)
    res2 = bass_utils.run_bass_kernel_spmd(nc2, in_maps2, core_ids, trace=TRACE)
    out = np.concatenate([res2.results[c]["out"][0] for c in range(NCORES)], axis=0)
    LAST_STATS["k1_ns"] = res1.exec_time_ns
    LAST_STATS["k2_ns"] = res2.exec_time_ns
    LAST_STATS["ks"] = ks_full
    LAST_STATS["new_lens"] = new_lens
    return out.astype(np.float32)


# revision 11
# speedup vs baseline: 1.0898x; 1.0898x over previous
"""Trainium2 Bass kernel for nn_CNN_RNN_88347477278730.

Pipeline (data-parallel over batch, 8 rows per core on 8 cores):
  kernel1 (device): chunked fp16 input projection (half hoisted, half
      interleaved into the early recurrence), then the 512-step
      select-policy GRUCell recurrence in full fp16 state with two 4-row
      batch streams for ILP; decisions batched into matmuls + is_gt at
      the end.
  host: compaction (gather kept tokens to the front), new_lens, Ldyn.
  kernel2 (device): compiled per dynamic sequence-length bucket t2
      (multiple of 32 >= max(new_lens)); chunked proj0, 2-layer GRU
      recurrence pipelined with a small lag, per-chunk proj1, Kim-CNN
      convs as shifted matmuls with compile-time pool windows, final
      linear.

All recurrence matmuls are gate-major (lhsT = weight tiles [K=128,
M=128], moving operand = h [K, batch]) so gate tensors land
partition-major where the elementwise engines are fast. The per-step
elementwise chain is 9 ops balanced across Vector/Scalar/GpSimd.
"""

import os
import subprocess
import sys
import tempfile

import numpy as np

# ---------------------------------------------------------------- constants
B, T, E, H, NF = 64, 512, 768, 256, 100
NCORES = 8
BPC = B // NCORES  # batch rows per core
KE = E // 128      # 6 K-tiles over the embedding dim
KH = H // 128      # 2 K-tiles over the hidden dim
GC = (3 * H) // 128  # 6 gate chunks (r: 0-1, z: 2-3, n: 4-5)

_F32 = None  # set lazily to mybir.dt.float32


# ------------------------------------------------------------- tile patch
def _apply_tile_patch():
    """This walrus build rejects >2 sem waits on one SP control instruction;
    split the TileContext tail drain into several drains of <=2 waits."""
    import concourse.tile as tile
    from concourse.vector_clock import ScopedClock, VectorClock

    if getattr(tile.TileContext, "_drain_split_patched", False):
        return

    def _patched(self, tick_clock, wait_clock):
        gc = tick_clock.global_clock
        n = len(gc)
        for start in range(0, n, 1):
            vec = [0] * n
            any_set = False
            for p in range(start, min(start + 1, n)):
                vec[p] = gc[p]
                any_set = any_set or vec[p] > 0
            if not any_set:
                continue
            d = self.nc.sync.drain()
            wait_clock.add_sem_waits(d.ins, ScopedClock({None: VectorClock(vec)}))
        self.nc.all_engine_barrier()
        assert self.sems is not None
        popped = self.nc._tile_sem_poison_stack.pop()
        assert popped is self._sem_poison
        self.nc.clear_and_free_semaphores(list(self.sems.allocated().values()))
        self.nc.all_engine_barrier()

    tile.TileContext._drain_and_barrier = _patched
    tile.TileContext._drain_split_patched = True


# ------------------------------------------------------------- gumbel (CPU)
def _gumbel_cpu():
    """jax.random.gumbel(key(42), (T-1, B, 2), f32) — computed in a CPU-jax
    subprocess so the accelerator backend is never involved (it must be
    bit-identical to the reference's CPU computation)."""
    path = os.path.join(tempfile.mkdtemp(), "gumbel.npy")
    code = (
        "import numpy as np, jax, jax.numpy as jnp\n"
        f"g = jax.random.gumbel(jax.random.key(42), ({T - 1}, {B}, 2), jnp.float32)\n"
        f"np.save({path!r}, np.asarray(g))\n"
    )
    env = dict(os.environ)
    env["TRN_TERMINAL_POOL_IPS"] = ""
    env["JAX_PLATFORMS"] = "cpu"
    extra = [p for p in sys.path if p and os.path.isdir(p)]
    env["PYTHONPATH"] = os.pathsep.join(extra)
    subprocess.run([sys.executable, "-c", code], env=env, check=True, capture_output=True)
    return np.load(path)


# ------------------------------------------------------------- host packing
def _pack_T(a2d):
    """[rows(=128*k), cols] -> [128, k, cols] weight-tile layout."""
    rows, cols = a2d.shape
    k = rows // 128
    return np.ascontiguousarray(a2d.reshape(k, 128, cols).transpose(1, 0, 2)).astype(np.float32)


def _pack_bias(b1d):
    """[128*k] -> [128, k]"""
    k = b1d.shape[0] // 128
    return np.ascontiguousarray(b1d.reshape(k, 128).T).astype(np.float32)


def _pack_embT(emb_rows, t_len=T):
    """[bpc, T, E] -> [KE, 128, bpc*T] (e-major tiles, free dims (b, t))."""
    bpc = emb_rows.shape[0]
    x = emb_rows.transpose(2, 0, 1).reshape(KE, 128, bpc * t_len)
    return np.ascontiguousarray(x).astype(np.float32)


def _pack_gru_weights(Wih, Whh, bih, bhh):
    """Returns (wihT, whhT, bias_proj, bhhn_rep) packings.

    bias_proj folds bih+bhh for the r,z chunks (added once at projection
    time); n chunks get bih only, with bhh_n applied per-step (it must be
    added to h@Whh_n *before* the r* multiply)."""
    wihT = _pack_T(np.ascontiguousarray(Wih.T))  # [128, KE or KH, 3H]
    whhT = _pack_T(np.ascontiguousarray(Whh.T))  # [128, KH, 3H]
    bias = np.empty(3 * H, np.float32)
    bias[: 2 * H] = bih[: 2 * H] + bhh[: 2 * H]
    bias[2 * H:] = bih[2 * H:]
    bias_proj = _pack_bias(bias)  # [128, GC]
    bhhn = _pack_bias(bhh[2 * H:])  # [128, KH]
    bhhn_rep = np.ascontiguousarray(
        np.broadcast_to(bhhn[:, :, None], (128, KH, BPC))
    ).astype(np.float32)
    return wihT, whhT, bias_proj, bhhn_rep


def _np_f16():
    from concourse import mybir

    return mybir.dt.np(mybir.dt.float16)


# ------------------------------------------------------------- bass builders
def _mk_nc():
    import concourse.bass as bass

    return bass.Bass("TRN2", target_bir_lowering=False, debug=False, num_devices=1)


def _split_excess_waits(nc, max_waits=1):
    """This walrus build can only encode ~2 sem waits per instruction
    (setupSyncWait 'Too many sync wait commands'). Hoist excess waits onto
    same-engine NoOps inserted just before the over-subscribed instruction;
    engine queues execute in order, so the wait semantics are identical."""
    from concourse import mybir

    nid = [0]
    for f in nc.m.functions:
        for bb in f.blocks:
            out = []
            changed = False
            for inst in bb.instructions:
                si = inst.sync_info
                lim = max_waits
                if si is not None and si.on_wait and len(si.on_wait) > lim:
                    waits = list(si.on_wait)
                    extra, keep = waits[:-lim], waits[-lim:]
                    for j in range(0, len(extra), max_waits):
                        nop = mybir.InstNoOp(
                            name=f"I-waitnop-{nid[0]}", ins=[], outs=[])
                        nid[0] += 1
                        nop.engine = inst.engine
                        nop.sync_info = mybir.SyncInfo(
                            on_wait=extra[j: j + max_waits], on_update=[])
                        nc.register_instruction(nop, overwrite=True)
                        out.append(nop)
                    inst.sync_info = mybir.SyncInfo(
                        on_wait=keep, on_update=list(si.on_update or []))
                    changed = True
                out.append(inst)
            if changed:
                bb.instructions = out
    return nc


def _proj_builder(nc, tc, misc_pool, dma_pool, big_ps, src_dram, wihT, biasc,
                  biasc_rep, gi, t_len, n_k, act, alu, f16, f32, tag):
    """Returns (emit_prefix, groups): chunked input projection.

    emit_prefix() emits the first t-half; `groups` is a list of closures,
    each emitting one (b, c) group of the second t-half (to be interleaved
    into the early recurrence steps). Copies alternate ACT (per-partition
    bias) / DVE (bias_rep tensor) to balance engines."""
    HALF = t_len // 2
    src_tiles = {}

    def _src(hb, b):
        key = (hb, b)
        if key not in src_tiles:
            s = dma_pool.tile([128, n_k, HALF], f16, tag=f"{tag}src")
            for k in range(n_k):
                nc.sync.dma_start(
                    s[:, k, :],
                    src_dram[k, :, b * t_len + hb * HALF: b * t_len + hb * HALF + HALF],
                )
            src_tiles[key] = s
        return src_tiles[key]

    def _group(hb, b, c):
        src = _src(hb, b)
        ps = big_ps.tile([128, HALF], f32, tag="bps")
        for k in range(n_k):
            nc.tensor.matmul(
                ps[:], wihT[:, k, c * 128: (c + 1) * 128], src[:, k, :],
                start=(k == 0), stop=(k == n_k - 1),
            )
        t0 = hb * HALF
        dst = gi[:, c, b, t0: t0 + HALF]
        if (b * GC + c) % 2 == 0:
            nc.scalar.activation(dst, ps[:], act.Identity, bias=biasc[:, c: c + 1])
        else:
            nc.vector.tensor_tensor(dst, ps[:], biasc_rep[:, c, :], alu.add)

    def emit_prefix():
        for b in range(BPC):
            for c in range(GC):
                _group(0, b, c)

    groups = [
        (lambda b=b, c=c: _group(1, b, c))
        for b in range(BPC) for c in range(GC)
    ]
    return emit_prefix, groups


def build_kernel1(t_len=T):
    """Select-policy kernel: fp16 everywhere; ONE merged 8-row stream (the
    per-step serial chain latency is the period — extra streams only add
    engine-queue coupling); gi_rz and bhh_n preloaded into PSUM off-chain so
    the gate matmuls accumulate straight onto them (start=False after a
    has_written-priming dummy matmul); 7-op chain; decisions batched at
    the end."""
    import concourse.tile as tile
    from concourse import mybir

    _apply_tile_patch()
    nc = _mk_nc()
    f32 = mybir.dt.float32
    f16 = mybir.dt.float16
    act = mybir.ActivationFunctionType
    alu = mybir.AluOpType
    HALF = t_len // 2

    embT_d = nc.dram_tensor("embT", [KE, 128, BPC * t_len], f16, kind="ExternalInput").ap()
    wihcT_d = nc.dram_tensor("wihcT", [128, KE, 3 * H], f16, kind="ExternalInput").ap()
    biasc_d = nc.dram_tensor("biasc", [128, GC], f32, kind="ExternalInput").ap()
    biascr_d = nc.dram_tensor("biascr", [128, GC, HALF], f32, kind="ExternalInput").ap()
    whh16_d = nc.dram_tensor("whh16", [128, KH, 3 * H], f16, kind="ExternalInput").ap()
    bhhnc_d = nc.dram_tensor("bhhnc", [128, KH, BPC], f32, kind="ExternalInput").ap()
    wdiffT_d = nc.dram_tensor("wdiffT", [128, KH, 1], f16, kind="ExternalInput").ap()
    ncdiff_d = nc.dram_tensor("ncdiff", [1, BPC * t_len], f32, kind="ExternalInput").ap()
    ks_d = nc.dram_tensor("ks", [1, BPC * t_len], f32, kind="ExternalOutput").ap()

    with tile.TileContext(nc) as tc:
        from contextlib import ExitStack

        with ExitStack() as ctx:
            wpool = ctx.enter_context(tc.tile_pool(name="weights", bufs=1))
            gipool = ctx.enter_context(tc.tile_pool(name="gi", bufs=1))
            hpool = ctx.enter_context(tc.tile_pool(name="hist", bufs=1))
            dma_pool = ctx.enter_context(tc.tile_pool(name="dma", bufs=2))
            big_ps = ctx.enter_context(tc.tile_pool(name="bigps", bufs=2, space="PSUM"))
            rz_psp = ctx.enter_context(tc.tile_pool(name="rzps", bufs=1, space="PSUM"))
            n_psp = ctx.enter_context(tc.tile_pool(name="nps", bufs=1, space="PSUM"))
            sb_pool = ctx.enter_context(tc.tile_pool(name="gates", bufs=2))
            misc = ctx.enter_context(tc.tile_pool(name="misc", bufs=1))

            def _load(pool, dram, shape, tag, dt=f32):
                t_ = pool.tile(shape, dt, tag=tag)
                nc.sync.dma_start(t_[:], dram[:])
                return t_

            wihcT = _load(wpool, wihcT_d, [128, KE, 3 * H], "wihcT", f16)
            biasc = _load(wpool, biasc_d, [128, GC], "biasc")
            biascr = _load(wpool, biascr_d, [128, GC, HALF], "biascr")
            whh16 = _load(wpool, whh16_d, [128, KH, 3 * H], "whh16", f16)
            bhhnc_rep = _load(wpool, bhhnc_d, [128, KH, BPC], "bhhnc")
            wdiffT = _load(misc, wdiffT_d, [128, KH, 1], "wdiffT", f16)
            ncdiff = _load(misc, ncdiff_d, [1, BPC * t_len], "ncdiff")
            zs = misc.tile([1, 128], f16, tag="zs")
            zx = misc.tile([1, 4 * BPC], f16, tag="zx")
            nc.vector.memset(zs[:], 0.0)
            nc.vector.memset(zx[:], 0.0)

            # gi layout: [128, GC, BPC, t] (t innermost: contiguous proj copies)
            gi = gipool.tile([128, GC, BPC, t_len], f16, tag="gi")
            emit_prefix, groups = _proj_builder(
                nc, tc, misc, dma_pool, big_ps, embT_d, wihcT, biasc, biascr,
                gi, t_len, KE, act, alu, f16, f32, "proj")
            emit_prefix()

            hist = hpool.tile([128, KH, BPC, t_len], f16, tag="hist")
            rz_ps = rz_psp.tile([128, 4, BPC], f32, tag="rz")
            n_ps = n_psp.tile([128, KH, BPC], f32, tag="n")

            # prime has_written for the preload+accumulate banks
            nc.tensor.matmul(rz_ps[:], zs[:], zx[:], start=True, stop=True)
            nc.tensor.matmul(n_ps[:], zs[:], zx[0:1, 0: KH * BPC], start=True, stop=True)
            # initial preloads for t=0
            nc.vector.tensor_copy(rz_ps[:], gi[:, 0:4, :, 0])
            nc.scalar.activation(n_ps[:], bhhnc_rep[:], act.Identity)

            def _gh(w, rhs, chunks, last=False):
                """Accumulate Whh @ rhs into the gate banks (start=False)."""
                for c in chunks:
                    dst = rz_ps[:, c, :] if c < 4 else n_ps[:, c - 4, :]
                    for k in range(KH):
                        nc.tensor.matmul(
                            dst, w[:, k, c * 128: (c + 1) * 128], rhs[:, k, :],
                            start=False, stop=(last and c == chunks[-1] and k == KH - 1),
                            skip_group_check=True,
                        )

            def emit_step(t):
                # gh(t) was already accumulated by step t-1 (MM-split: the
                # Whh@nn part right after tanh, the Whh@d part after op8).
                # Chain: sigma_r -> x r -> +gi_n -> tanh -> (h_prev-nn) ->
                # x z -> MM_d; sigma_z, preloads, MM_nn, h-write off-chain.
                rz = sb_pool.tile([128, 4, BPC], f32, tag="rz")
                tmp = sb_pool.tile([128, KH, BPC], f32, tag="tmp")
                nc.scalar.activation(rz[:, 0:2, :], rz_ps[:, 0:2, :], act.Sigmoid)
                nc.scalar.activation(rz[:, 2:4, :], rz_ps[:, 2:4, :], act.Sigmoid)
                nc.vector.tensor_tensor(tmp[:], n_ps[:], rz[:, 0:KH, :], alu.mult)
                nc.vector.tensor_tensor(tmp[:], tmp[:], gi[:, 4:GC, :, t], alu.add)
                if t + 1 < t_len:
                    nc.vector.tensor_copy(rz_ps[:], gi[:, 0:4, :, t + 1])
                nn16 = sb_pool.tile([128, KH, BPC], f16, tag="nn")
                nc.scalar.activation(nn16[:], tmp[:], act.Tanh)
                if t + 1 < t_len:
                    nc.scalar.activation(n_ps[:], bhhnc_rep[:], act.Identity)
                    _gh(whh16, nn16, (0, 1, 2, 3, 4, 5))
                d = sb_pool.tile([128, KH, BPC], f32, tag="dd")
                if t > 0:
                    nc.vector.tensor_tensor(d[:], hist[:, :, :, t - 1], nn16[:], alu.subtract)
                else:
                    nc.vector.tensor_scalar(d[:], nn16[:], -1.0, None, alu.mult)
                d16 = sb_pool.tile([128, KH, BPC], f16, tag="dd16")
                nc.vector.tensor_tensor(d16[:], rz[:, 2:4, :], d[:], alu.mult)
                if t + 1 < t_len:
                    _gh(whh16, d16, (0, 1, 2, 3, 4, 5), last=True)
                nc.gpsimd.tensor_tensor(hist[:, :, :, t], nn16[:], d16[:], alu.add)

            gidx = 0
            for t in range(t_len):
                emit_step(t)
                if t % 2 == 1 and gidx < len(groups):
                    groups[gidx]()
                    gidx += 1
            while gidx < len(groups):
                groups[gidx]()
                gidx += 1

            # ---- batched decisions: ks[b,t] = (h_t . wdiff > ncdiff) ----
            ks_sb = misc.tile([1, BPC * t_len], f32, tag="kssb")
            for b in range(BPC):
                dps = big_ps.tile([1, t_len], f32, tag="bps")
                for k in range(KH):
                    nc.tensor.matmul(
                        dps[:], wdiffT[:, k, :], hist[:, k, b, :],
                        start=(k == 0), stop=(k == KH - 1),
                    )
                nc.vector.tensor_tensor(
                    ks_sb[0:1, b * t_len: (b + 1) * t_len], dps[:],
                    ncdiff[0:1, b * t_len: (b + 1) * t_len], alu.is_gt,
                )
            nc.sync.dma_start(ks_d[:], ks_sb[:])

    return _split_excess_waits(nc)


def build_kernel2(t2, kf3, kf4, kf5):
    """GRU0/GRU1 + convs + pooling + final linear at dynamic length t2.

    Both layers use the k1-style low-latency step (PSUM preloads, split
    sigma, DVE-resident chain); no matmul split (two chains share the PE,
    so the per-wave PE budget matters more than each chain's MM segment).
    proj1 computed per-D-chunk from the o1 history into a gi1 buffer;
    conv max-pool windows (kf*) are compile-time constants."""
    import concourse.tile as tile
    from concourse import mybir

    _apply_tile_patch()
    nc = _mk_nc()
    f32 = mybir.dt.float32
    f16 = mybir.dt.float16
    act = mybir.ActivationFunctionType
    alu = mybir.AluOpType
    D = 32
    LAG = D + 8
    HALF = t2 // 2

    nembT_d = nc.dram_tensor("nembT", [KE, 128, BPC * t2], f16, kind="ExternalInput").ap()
    wih0T_d = nc.dram_tensor("wih0T", [128, KE, 3 * H], f16, kind="ExternalInput").ap()
    whh0T_d = nc.dram_tensor("whh0T", [128, KH, 3 * H], f16, kind="ExternalInput").ap()
    bias0_d = nc.dram_tensor("bias0", [128, GC], f32, kind="ExternalInput").ap()
    bias0r_d = nc.dram_tensor("bias0r", [128, GC, HALF], f32, kind="ExternalInput").ap()
    bhhn0_d = nc.dram_tensor("bhhn0", [128, KH, BPC], f32, kind="ExternalInput").ap()
    wih1T_d = nc.dram_tensor("wih1T", [128, KH, 3 * H], f16, kind="ExternalInput").ap()
    whh1T_d = nc.dram_tensor("whh1T", [128, KH, 3 * H], f16, kind="ExternalInput").ap()
    bias1_d = nc.dram_tensor("bias1", [128, GC], f32, kind="ExternalInput").ap()
    bias1r_d = nc.dram_tensor("bias1r", [128, GC, D], f32, kind="ExternalInput").ap()
    bhhn1_d = nc.dram_tensor("bhhn1", [128, KH, BPC], f32, kind="ExternalInput").ap()
    vt_d = nc.dram_tensor("vt", [1, BPC * t2], f16, kind="ExternalInput").ap()
    cw_d = nc.dram_tensor("cw", [128, 12, KH, NF], f16, kind="ExternalInput").ap()
    cb_d = nc.dram_tensor("cb", [NF, 3], f32, kind="ExternalInput").ap()
    woutT_d = nc.dram_tensor("woutT", [NF, 3], f32, kind="ExternalInput").ap()
    bout_d = nc.dram_tensor("bout", [1, 1], f32, kind="ExternalInput").ap()
    out_d = nc.dram_tensor("out", [1, BPC], f32, kind="ExternalOutput").ap()

    FS = (3, 4, 5)
    KFS = (kf3, kf4, kf5)

    with tile.TileContext(nc) as tc:
        from contextlib import ExitStack

        with ExitStack() as ctx:
            wpool = ctx.enter_context(tc.tile_pool(name="weights", bufs=1))
            gipool = ctx.enter_context(tc.tile_pool(name="gi", bufs=1))
            opool = ctx.enter_context(tc.tile_pool(name="obuf", bufs=1))
            dma_pool = ctx.enter_context(tc.tile_pool(name="dma", bufs=2))
            big_ps = ctx.enter_context(tc.tile_pool(name="bigps", bufs=2, space="PSUM"))
            rz0_psp = ctx.enter_context(tc.tile_pool(name="rz0ps", bufs=1, space="PSUM"))
            n0_psp = ctx.enter_context(tc.tile_pool(name="n0ps", bufs=1, space="PSUM"))
            rz1_psp = ctx.enter_context(tc.tile_pool(name="rz1ps", bufs=1, space="PSUM"))
            n1_psp = ctx.enter_context(tc.tile_pool(name="n1ps", bufs=1, space="PSUM"))
            sb_pool = ctx.enter_context(tc.tile_pool(name="gates", bufs=2))
            misc = ctx.enter_context(tc.tile_pool(name="misc", bufs=1))

            def _load(pool, dram, shape, tag, dt=f32):
                t_ = pool.tile(shape, dt, tag=tag)
                nc.sync.dma_start(t_[:], dram[:])
                return t_

            wih0T = _load(wpool, wih0T_d, [128, KE, 3 * H], "wih0", f16)
            whh0T = _load(wpool, whh0T_d, [128, KH, 3 * H], "whh0", f16)
            bias0 = _load(wpool, bias0_d, [128, GC], "bias0")
            bias0r = _load(wpool, bias0r_d, [128, GC, HALF], "bias0r")
            bhhn0 = _load(wpool, bhhn0_d, [128, KH, BPC], "bhhn0")
            wih1T = _load(wpool, wih1T_d, [128, KH, 3 * H], "wih1", f16)
            whh1T = _load(wpool, whh1T_d, [128, KH, 3 * H], "whh1", f16)
            bias1 = _load(wpool, bias1_d, [128, GC], "bias1")
            bias1r = _load(wpool, bias1r_d, [128, GC, D], "bias1r")
            bhhn1 = _load(wpool, bhhn1_d, [128, KH, BPC], "bhhn1")
            cw = _load(wpool, cw_d, [128, 12, KH, NF], "cw", f16)
            cb = _load(misc, cb_d, [NF, 3], "cb")
            woutT = _load(misc, woutT_d, [NF, 3], "woutT")
            bout = _load(misc, bout_d, [1, 1], "bout")
            vt = _load(misc, vt_d, [1, BPC * t2], "vt", f16)
            zs = misc.tile([1, 128], f16, tag="zs")
            zx = misc.tile([1, 4 * BPC], f16, tag="zx")
            nc.vector.memset(zs[:], 0.0)
            nc.vector.memset(zx[:], 0.0)

            gi0 = gipool.tile([128, GC, BPC, t2], f16, tag="gi0")
            gi1 = gipool.tile([128, GC, BPC, t2], f16, tag="gi1")
            o1 = opool.tile([128, KH, BPC, t2], f16, tag="o1")
            o2 = opool.tile([128, KH, BPC, t2], f16, tag="o2")

            emit_prefix, groups = _proj_builder(
                nc, tc, misc, dma_pool, big_ps, nembT_d, wih0T, bias0, bias0r,
                gi0, t2, KE, act, alu, f16, f32, "proj0")
            emit_prefix()

            rz0 = rz0_psp.tile([128, 4, BPC], f32, tag="rz0")
            n0 = n0_psp.tile([128, KH, BPC], f32, tag="n0")
            rz1 = rz1_psp.tile([128, 4, BPC], f32, tag="rz1")
            n1 = n1_psp.tile([128, KH, BPC], f32, tag="n1")

            for ps_t in (rz0, rz1):
                nc.tensor.matmul(ps_t[:], zs[:], zx[:], start=True, stop=True)
            for ps_t in (n0, n1):
                nc.tensor.matmul(ps_t[:], zs[:], zx[0:1, 0: KH * BPC], start=True, stop=True)
            # initial preloads
            nc.vector.tensor_copy(rz0[:], gi0[:, 0:4, :, 0])
            nc.scalar.activation(n0[:], bhhn0[:], act.Identity)
            nc.scalar.activation(n1[:], bhhn1[:], act.Identity)

            def emit_step(t, gi, whh, bhhn, hist, prev, rz_ps, n_ps, sfx, tl):
                """One GRU layer step, matmul-split form: gh(t+1) is
                accumulated as Whh@nn16 (after tanh) + Whh@d16 (after the
                z-mult), so the h-write and the full burst leave the chain."""
                def gh(rhs, last):
                    for c in (0, 1, 2, 3, 4, 5):
                        dst = rz_ps[:, c, :] if c < 4 else n_ps[:, c - 4, :]
                        for k in range(KH):
                            nc.tensor.matmul(
                                dst, whh[:, k, c * 128: (c + 1) * 128],
                                rhs[:, k, :],
                                start=False, stop=(last and k == KH - 1),
                                skip_group_check=True,
                            )
                rz = sb_pool.tile([128, 4, BPC], f32, tag="rz" + sfx)
                tmp = sb_pool.tile([128, KH, BPC], f32, tag="tmp" + sfx)
                nc.scalar.activation(rz[:], rz_ps[:], act.Sigmoid)
                nc.vector.tensor_tensor(tmp[:], n_ps[:], rz[:, 0:KH, :], alu.mult)
                if t + 1 < tl:
                    nc.vector.tensor_copy(rz_ps[:], gi[:, 0:4, :, t + 1])
                nc.gpsimd.tensor_tensor(tmp[:], tmp[:], gi[:, 4:GC, :, t], alu.add)
                nn16 = sb_pool.tile([128, KH, BPC], f16, tag="nn" + sfx)
                nc.scalar.activation(nn16[:], tmp[:], act.Tanh)
                if t + 1 < tl:
                    nc.scalar.activation(n_ps[:], bhhn[:], act.Identity)
                    gh(nn16, last=False)
                d = sb_pool.tile([128, KH, BPC], f32, tag="dd" + sfx)
                if t > 0:
                    nc.vector.tensor_tensor(d[:], hist[:, :, :, t - 1], nn16[:], alu.subtract)
                else:
                    nc.vector.tensor_scalar(d[:], nn16[:], -1.0, None, alu.mult)
                d16 = sb_pool.tile([128, KH, BPC], f16, tag="d6" + sfx)
                nc.vector.tensor_tensor(d16[:], rz[:, 2:4, :], d[:], alu.mult)
                if t + 1 < tl:
                    gh(d16, last=True)
                nc.gpsimd.tensor_tensor(hist[:, :, :, t], nn16[:], d16[:], alu.add)

            def emit_proj1_chunk(ci):
                t0, t1 = ci * D, (ci + 1) * D
                for b in range(BPC):
                    for c in range(GC):
                        ps = big_ps.tile([128, D], f32, tag="bps")
                        for k in range(KH):
                            nc.tensor.matmul(
                                ps[:], wih1T[:, k, c * 128: (c + 1) * 128],
                                o1[:, k, b, t0:t1],
                                start=(k == 0), stop=(k == KH - 1),
                            )
                        dst = gi1[:, c, b, t0:t1]
                        if (b * GC + c) % 2 == 0:
                            nc.scalar.activation(
                                dst, ps[:], act.Identity, bias=bias1[:, c: c + 1])
                        else:
                            nc.vector.tensor_tensor(dst, ps[:], bias1r[:, c, :], alu.add)

            l1_started = [False]
            gidx = 0
            for w in range(t2 + LAG):
                if w < t2:
                    emit_step(w, gi0, whh0T, bhhn0, o1, o1, rz0, n0, "0", t2)
                    if w % 2 == 1 and gidx < len(groups):
                        groups[gidx]()
                        gidx += 1
                if w >= LAG:
                    t = w - LAG
                    if not l1_started[0]:
                        nc.vector.tensor_copy(rz1[:], gi1[:, 0:4, :, 0])
                        l1_started[0] = True
                    emit_step(t, gi1, whh1T, bhhn1, o2, o2, rz1, n1, "1", t2)
                if w < t2 and w % D == D - 1:
                    emit_proj1_chunk(w // D)
            while gidx < len(groups):
                groups[gidx]()
                gidx += 1

            # ---- zero o2 past new_lens: o2 *= vt ----
            # partition-broadcast vt via a K=1 ones-matmul (PE outer product)
            ones_sb = misc.tile([1, 128], f16, tag="ones")
            nc.vector.memset(ones_sb[:], 1.0)
            for b in range(BPC):
                vtb = big_ps.tile([128, t2], f32, tag="bps")
                nc.tensor.matmul(
                    vtb[:], ones_sb[:], vt[0:1, b * t2: (b + 1) * t2],
                    start=True, stop=True,
                )
                for k in range(KH):
                    nc.vector.tensor_tensor(
                        o2[:, k, b, :], o2[:, k, b, :], vtb[:], alu.mult
                    )

            # ---- convs + relu + max-pool over compile-time window ----
            pooled = misc.tile([NF, 3, BPC], f32, tag="pooled")
            for b in range(BPC):
                for fi, fs in enumerate(FS):
                    nw = t2 - fs + 1
                    kf = KFS[fi]
                    ps = big_ps.tile([NF, t2], f32, tag="bps")
                    m0 = sum(FS[:fi])  # flat (fs,dt) base index
                    first = True
                    for dt_ in range(fs):
                        for k in range(KH):
                            nc.tensor.matmul(
                                ps[:, :nw],
                                cw[:, m0 + dt_, k, :],
                                o2[:, k, b, dt_: dt_ + nw],
                                start=first,
                                stop=(dt_ == fs - 1 and k == KH - 1),
                            )
                            first = False
                    crelu = sb_pool.tile([NF, t2], f32, tag="crelu")
                    nc.scalar.activation(
                        crelu[:, :kf], ps[:, :kf], act.Relu, bias=cb[:, fi: fi + 1]
                    )
                    nc.vector.tensor_reduce(
                        pooled[:, fi, b: b + 1], crelu[:, :kf],
                        mybir.AxisListType.X, alu.max,
                    )

            # ---- final linear ----
            fps = big_ps.tile([1, BPC], f32, tag="bps")
            for fi in range(3):
                nc.tensor.matmul(
                    fps[:], woutT[:, fi: fi + 1], pooled[:, fi, :],
                    start=(fi == 0), stop=(fi == 2),
                )
            out_sb = misc.tile([1, BPC], f32, tag="outsb")
            nc.scalar.activation(out_sb[:], fps[:], act.Identity, bias=bout[0:1, 0:1])
            nc.sync.dma_start(out_d[:], out_sb[:])

    return _split_excess_waits(nc)


# ------------------------------------------------------------- host orchestration
def _host_pack_k1(inputs, gumbel, t_len=T):
    f16 = _np_f16()
    emb = np.asarray(inputs["embedded"], np.float32)
    mask = np.asarray(inputs["mask"])
    lens = mask.sum(1)
    maxlen = int(lens.max())

    wihcT, whhT, biasc, bhhnc = _pack_gru_weights(
        inputs["Wih_c"], inputs["Whh_c"], inputs["bih_c"], inputs["bhh_c"])
    wdiff = (inputs["Wsel"][1] - inputs["Wsel"][0]).astype(np.float32)
    wdiffT = np.ascontiguousarray(wdiff.reshape(KH, 128).T[:, :, None])
    bdiff = float(inputs["bsel"][1] - inputs["bsel"][0])

    # ncdiff[b, t]: k_t = (h.wdiff > ncdiff); forced off when t >= maxlen-1
    ncdiff = np.full((B, t_len), 1.0e30, np.float32)
    upto = min(maxlen - 1, t_len)
    for t in range(1, upto):
        ncdiff[:, t] = -(bdiff + gumbel[t - 1, :, 1] - gumbel[t - 1, :, 0])

    biascr = np.ascontiguousarray(
        np.broadcast_to(biasc[:, :, None], (128, GC, t_len // 2))).astype(np.float32)

    in_maps = []
    for c in range(NCORES):
        rows = slice(c * BPC, (c + 1) * BPC)
        in_maps.append({
            "embT": _pack_embT(emb[rows, :t_len], t_len).astype(f16),
            "wihcT": wihcT.astype(f16),
            "biasc": biasc,
            "biascr": biascr,
            "whh16": whhT.astype(f16),
            "bhhnc": bhhnc,
            "wdiffT": wdiffT.astype(f16),
            "ncdiff": np.ascontiguousarray(
                ncdiff[rows].reshape(1, BPC * t_len)),
        })
    return in_maps, lens, maxlen


def _host_compact(inputs, ks_full, lens, maxlen, t_len=T):
    """ks_full: [B, t_len] decision bits (row t=0 ignored; selected[:,0]=1)."""
    emb = np.asarray(inputs["embedded"], np.float32)
    selected = np.zeros((B, t_len), np.int64)
    selected[:, 0] = 1
    selected[:, 1:] = ks_full[:, 1:]
    pos = np.arange(t_len)
    sel_valid = np.where(pos[None, :] < (lens - 1)[:, None], selected, 0)
    new_mask = np.where(pos[None, :] == (lens - 1)[:, None], 1, sel_valid)
    new_lens = new_mask.sum(1)
    Ldyn = max(int(new_lens.max()), 7)

    t2 = max(-(-Ldyn // 64) * 64, 64)
    new_emb = np.zeros((B, t2, E), np.float32)
    for b in range(B):
        idx = np.nonzero(new_mask[b])[0]
        new_emb[b, : len(idx)] = emb[b, idx]
    return new_emb, new_lens, Ldyn, t2


def _host_pack_k2(inputs, new_emb, new_lens, Ldyn, t2):
    f16 = _np_f16()
    wih0T, whh0T, bias0, bhhn0 = _pack_gru_weights(
        inputs["Wih0"], inputs["Whh0"], inputs["bih0"], inputs["bhh0"])
    wih1T, whh1T, bias1, bhhn1 = _pack_gru_weights(
        inputs["Wih1"], inputs["Whh1"], inputs["bih1"], inputs["bhh1"])

    FS = (3, 4, 5)
    cw = np.zeros((128, 12, KH, NF), np.float32)
    cb = np.zeros((NF, 3), np.float32)
    m = 0
    for fi, fs in enumerate(FS):
        w = np.asarray(inputs[f"conv_w{fs}"], np.float32)  # [NF,1,fs,H]
        cb[:, fi] = np.asarray(inputs[f"conv_b{fs}"], np.float32)
        for dt_ in range(fs):
            wt = w[:, 0, dt_, :].T  # [H, NF]
            cw[:, m, :, :] = wt.reshape(KH, 128, NF).transpose(1, 0, 2)
            m += 1

    woutT = np.ascontiguousarray(
        np.asarray(inputs["Wout"], np.float32)[0].reshape(3, NF).T)
    bout = np.asarray(inputs["bout"], np.float32).reshape(1, 1)

    vt_full = (np.arange(t2)[None, :] < new_lens[:, None]).astype(np.float32)

    bias0r = np.ascontiguousarray(
        np.broadcast_to(bias0[:, :, None], (128, GC, t2 // 2))).astype(np.float32)
    bias1r = np.ascontiguousarray(
        np.broadcast_to(bias1[:, :, None], (128, GC, 32))).astype(np.float32)

    in_maps = []
    for c in range(NCORES):
        rows = slice(c * BPC, (c + 1) * BPC)
        in_maps.append({
            "nembT": _pack_embT(new_emb[rows], t2).astype(f16),
            "wih0T": wih0T.astype(f16), "whh0T": whh0T.astype(f16),
            "bias0": bias0, "bias0r": bias0r, "bhhn0": bhhn0,
            "wih1T": wih1T.astype(f16), "whh1T": whh1T.astype(f16),
            "bias1": bias1, "bias1r": bias1r, "bhhn1": bhhn1,
            "vt": np.ascontiguousarray(
                vt_full[rows].reshape(1, BPC * t2)).astype(f16),
            "cw": cw.astype(f16), "cb": cb,
            "woutT": woutT, "bout": bout,
        })
    return in_maps


_NC_CACHE = {}


def _get_nc1(t_len=T):
    key = (1, t_len)
    if key not in _NC_CACHE:
        _NC_CACHE[key] = build_kernel1(t_len)
    return _NC_CACHE[key]


def _get_nc2(t2, kfs):
    key = (2, t2, kfs)
    if key not in _NC_CACHE:
        _NC_CACHE[key] = build_kernel2(t2, *kfs)
    return _NC_CACHE[key]


TRACE = False  # set True (with an NTFF hook registered) to collect exec times
LAST_STATS = {}


def kernel(**inputs):
    from concourse import bass_utils

    gumbel = _gumbel_cpu()
    core_ids = list(range(NCORES))

    in_maps1, lens, maxlen = _host_pack_k1(inputs, gumbel)
    nc1 = _get_nc1()
    res1 = bass_utils.run_bass_kernel_spmd(nc1, in_maps1, core_ids, trace=TRACE)
    ks_full = np.concatenate(
        [res1.results[c]["ks"].reshape(BPC, T) for c in range(NCORES)], axis=0)

    new_emb, new_lens, Ldyn, t2 = _host_compact(inputs, ks_full, lens, maxlen)
    kfs = tuple(min(Ldyn - fs + 1, t2 - fs + 1) for fs in (3, 4, 5))
    in_maps2 = _host_pack_k2(inputs, new_emb, new_lens, Ldyn, t2)
    nc2 = _get_nc2(t2, kfs)
    res2 = bass_utils.run_bass_kernel_spmd(nc2, in_maps2, core_ids, trace=TRACE)
    out = np.concatenate([res2.results[c]["out"][0] for c in range(NCORES)], axis=0)
    LAST_STATS["k1_ns"] = res1.exec_time_ns
    LAST_STATS["k2_ns"] = res2.exec_time_ns
    LAST_STATS["ks"] = ks_full
    LAST_STATS["new_lens"] = new_lens
    return out.astype(np.float32)


# revision 12
# speedup vs baseline: 1.1008x; 1.0101x over previous
"""Trainium2 Bass kernel for nn_CNN_RNN_88347477278730.

Pipeline (data-parallel over batch, 8 rows per core on 8 cores):
  kernel1 (device): chunked fp16 input projection (half hoisted, half
      interleaved into the early recurrence), then the 512-step
      select-policy GRUCell recurrence in full fp16 state with two 4-row
      batch streams for ILP; decisions batched into matmuls + is_gt at
      the end.
  host: compaction (gather kept tokens to the front), new_lens, Ldyn.
  kernel2 (device): compiled per dynamic sequence-length bucket t2
      (multiple of 32 >= max(new_lens)); chunked proj0, 2-layer GRU
      recurrence pipelined with a small lag, per-chunk proj1, Kim-CNN
      convs as shifted matmuls with compile-time pool windows, final
      linear.

All recurrence matmuls are gate-major (lhsT = weight tiles [K=128,
M=128], moving operand = h [K, batch]) so gate tensors land
partition-major where the elementwise engines are fast. The per-step
elementwise chain is 9 ops balanced across Vector/Scalar/GpSimd.
"""

import os
import subprocess
import sys
import tempfile

import numpy as np

# ---------------------------------------------------------------- constants
B, T, E, H, NF = 64, 512, 768, 256, 100
NCORES = 8
BPC = B // NCORES  # batch rows per core
KE = E // 128      # 6 K-tiles over the embedding dim
KH = H // 128      # 2 K-tiles over the hidden dim
GC = (3 * H) // 128  # 6 gate chunks (r: 0-1, z: 2-3, n: 4-5)

_F32 = None  # set lazily to mybir.dt.float32


# ------------------------------------------------------------- tile patch
def _apply_tile_patch():
    """This walrus build rejects >2 sem waits on one SP control instruction;
    split the TileContext tail drain into several drains of <=2 waits."""
    import concourse.tile as tile
    from concourse.vector_clock import ScopedClock, VectorClock

    if getattr(tile.TileContext, "_drain_split_patched", False):
        return

    def _patched(self, tick_clock, wait_clock):
        gc = tick_clock.global_clock
        n = len(gc)
        for start in range(0, n, 1):
            vec = [0] * n
            any_set = False
            for p in range(start, min(start + 1, n)):
                vec[p] = gc[p]
                any_set = any_set or vec[p] > 0
            if not any_set:
                continue
            d = self.nc.sync.drain()
            wait_clock.add_sem_waits(d.ins, ScopedClock({None: VectorClock(vec)}))
        self.nc.all_engine_barrier()
        assert self.sems is not None
        popped = self.nc._tile_sem_poison_stack.pop()
        assert popped is self._sem_poison
        self.nc.clear_and_free_semaphores(list(self.sems.allocated().values()))
        self.nc.all_engine_barrier()

    tile.TileContext._drain_and_barrier = _patched
    tile.TileContext._drain_split_patched = True


# ------------------------------------------------------------- gumbel (CPU)
def _gumbel_cpu():
    """jax.random.gumbel(key(42), (T-1, B, 2), f32) — computed in a CPU-jax
    subprocess so the accelerator backend is never involved (it must be
    bit-identical to the reference's CPU computation)."""
    path = os.path.join(tempfile.mkdtemp(), "gumbel.npy")
    code = (
        "import numpy as np, jax, jax.numpy as jnp\n"
        f"g = jax.random.gumbel(jax.random.key(42), ({T - 1}, {B}, 2), jnp.float32)\n"
        f"np.save({path!r}, np.asarray(g))\n"
    )
    env = dict(os.environ)
    env["TRN_TERMINAL_POOL_IPS"] = ""
    env["JAX_PLATFORMS"] = "cpu"
    extra = [p for p in sys.path if p and os.path.isdir(p)]
    env["PYTHONPATH"] = os.pathsep.join(extra)
    subprocess.run([sys.executable, "-c", code], env=env, check=True, capture_output=True)
    return np.load(path)


# ------------------------------------------------------------- host packing
def _pack_T(a2d):
    """[rows(=128*k), cols] -> [128, k, cols] weight-tile layout."""
    rows, cols = a2d.shape
    k = rows // 128
    return np.ascontiguousarray(a2d.reshape(k, 128, cols).transpose(1, 0, 2)).astype(np.float32)


def _pack_bias(b1d):
    """[128*k] -> [128, k]"""
    k = b1d.shape[0] // 128
    return np.ascontiguousarray(b1d.reshape(k, 128).T).astype(np.float32)


def _pack_embT(emb_rows, t_len=T):
    """[bpc, T, E] -> [KE, 128, bpc*T] (e-major tiles, free dims (b, t))."""
    bpc = emb_rows.shape[0]
    x = emb_rows.transpose(2, 0, 1).reshape(KE, 128, bpc * t_len)
    return np.ascontiguousarray(x).astype(np.float32)


def _pack_gru_weights(Wih, Whh, bih, bhh):
    """Returns (wihT, whhT, bias_proj, bhhn_rep) packings.

    bias_proj folds bih+bhh for the r,z chunks (added once at projection
    time); n chunks get bih only, with bhh_n applied per-step (it must be
    added to h@Whh_n *before* the r* multiply)."""
    wihT = _pack_T(np.ascontiguousarray(Wih.T))  # [128, KE or KH, 3H]
    whhT = _pack_T(np.ascontiguousarray(Whh.T))  # [128, KH, 3H]
    bias = np.empty(3 * H, np.float32)
    bias[: 2 * H] = bih[: 2 * H] + bhh[: 2 * H]
    bias[2 * H:] = bih[2 * H:]
    bias_proj = _pack_bias(bias)  # [128, GC]
    bhhn = _pack_bias(bhh[2 * H:])  # [128, KH]
    bhhn_rep = np.ascontiguousarray(
        np.broadcast_to(bhhn[:, :, None], (128, KH, BPC))
    ).astype(np.float32)
    return wihT, whhT, bias_proj, bhhn_rep


def _np_f16():
    from concourse import mybir

    return mybir.dt.np(mybir.dt.float16)


# ------------------------------------------------------------- bass builders
def _mk_nc():
    import concourse.bass as bass

    return bass.Bass("TRN2", target_bir_lowering=False, debug=False, num_devices=1)


def _split_excess_waits(nc, max_waits=1):
    """This walrus build can only encode ~2 sem waits per instruction
    (setupSyncWait 'Too many sync wait commands'). Hoist excess waits onto
    same-engine NoOps inserted just before the over-subscribed instruction;
    engine queues execute in order, so the wait semantics are identical."""
    from concourse import mybir

    nid = [0]
    for f in nc.m.functions:
        for bb in f.blocks:
            out = []
            changed = False
            for inst in bb.instructions:
                si = inst.sync_info
                lim = max_waits
                if si is not None and si.on_wait and len(si.on_wait) > lim:
                    waits = list(si.on_wait)
                    extra, keep = waits[:-lim], waits[-lim:]
                    for j in range(0, len(extra), max_waits):
                        nop = mybir.InstNoOp(
                            name=f"I-waitnop-{nid[0]}", ins=[], outs=[])
                        nid[0] += 1
                        nop.engine = inst.engine
                        nop.sync_info = mybir.SyncInfo(
                            on_wait=extra[j: j + max_waits], on_update=[])
                        nc.register_instruction(nop, overwrite=True)
                        out.append(nop)
                    inst.sync_info = mybir.SyncInfo(
                        on_wait=keep, on_update=list(si.on_update or []))
                    changed = True
                out.append(inst)
            if changed:
                bb.instructions = out
    return nc


def _proj_builder(nc, tc, misc_pool, dma_pool, big_ps, src_dram, wihT, biasc,
                  biasc_rep, gi, t_len, n_k, act, alu, f16, f32, tag):
    """Returns (emit_prefix, groups): chunked input projection.

    emit_prefix() emits the first t-half; `groups` is a list of closures,
    each emitting one (b, c) group of the second t-half (to be interleaved
    into the early recurrence steps). Copies alternate ACT (per-partition
    bias) / DVE (bias_rep tensor) to balance engines."""
    HALF = t_len // 2
    src_tiles = {}

    def _src(hb, b):
        key = (hb, b)
        if key not in src_tiles:
            s = dma_pool.tile([128, n_k, HALF], f16, tag=f"{tag}src")
            for k in range(n_k):
                nc.sync.dma_start(
                    s[:, k, :],
                    src_dram[k, :, b * t_len + hb * HALF: b * t_len + hb * HALF + HALF],
                )
            src_tiles[key] = s
        return src_tiles[key]

    def _group(hb, b, c):
        src = _src(hb, b)
        ps = big_ps.tile([128, HALF], f32, tag="bps")
        for k in range(n_k):
            nc.tensor.matmul(
                ps[:], wihT[:, k, c * 128: (c + 1) * 128], src[:, k, :],
                start=(k == 0), stop=(k == n_k - 1),
            )
        t0 = hb * HALF
        dst = gi[:, c, b, t0: t0 + HALF]
        if (b * GC + c) % 2 == 0:
            nc.scalar.activation(dst, ps[:], act.Identity, bias=biasc[:, c: c + 1])
        else:
            nc.vector.tensor_tensor(dst, ps[:], biasc_rep[:, c, :], alu.add)

    def emit_prefix():
        for b in range(BPC):
            for c in range(GC):
                _group(0, b, c)

    groups = [
        (lambda b=b, c=c: _group(1, b, c))
        for b in range(BPC) for c in range(GC)
    ]
    return emit_prefix, groups


def build_kernel1(t_len=T):
    """Select-policy kernel: fp16 everywhere; ONE merged 8-row stream (the
    per-step serial chain latency is the period — extra streams only add
    engine-queue coupling); gi_rz and bhh_n preloaded into PSUM off-chain so
    the gate matmuls accumulate straight onto them (start=False after a
    has_written-priming dummy matmul); 7-op chain; decisions batched at
    the end."""
    import concourse.tile as tile
    from concourse import mybir

    _apply_tile_patch()
    nc = _mk_nc()
    f32 = mybir.dt.float32
    f16 = mybir.dt.float16
    act = mybir.ActivationFunctionType
    alu = mybir.AluOpType
    HALF = t_len // 2

    embT_d = nc.dram_tensor("embT", [KE, 128, BPC * t_len], f16, kind="ExternalInput").ap()
    wihcT_d = nc.dram_tensor("wihcT", [128, KE, 3 * H], f16, kind="ExternalInput").ap()
    biasc_d = nc.dram_tensor("biasc", [128, GC], f32, kind="ExternalInput").ap()
    biascr_d = nc.dram_tensor("biascr", [128, GC, HALF], f32, kind="ExternalInput").ap()
    whh16_d = nc.dram_tensor("whh16", [128, KH, 3 * H], f16, kind="ExternalInput").ap()
    bhhnc_d = nc.dram_tensor("bhhnc", [128, KH, BPC], f32, kind="ExternalInput").ap()
    wdiffT_d = nc.dram_tensor("wdiffT", [128, KH, 1], f16, kind="ExternalInput").ap()
    ncdiff_d = nc.dram_tensor("ncdiff", [1, BPC * t_len], f32, kind="ExternalInput").ap()
    ks_d = nc.dram_tensor("ks", [1, BPC * t_len], f32, kind="ExternalOutput").ap()

    with tile.TileContext(nc) as tc:
        from contextlib import ExitStack

        with ExitStack() as ctx:
            wpool = ctx.enter_context(tc.tile_pool(name="weights", bufs=1))
            gipool = ctx.enter_context(tc.tile_pool(name="gi", bufs=1))
            hpool = ctx.enter_context(tc.tile_pool(name="hist", bufs=1))
            dma_pool = ctx.enter_context(tc.tile_pool(name="dma", bufs=2))
            big_ps = ctx.enter_context(tc.tile_pool(name="bigps", bufs=2, space="PSUM"))
            rz_psp = ctx.enter_context(tc.tile_pool(name="rzps", bufs=1, space="PSUM"))
            n_psp = ctx.enter_context(tc.tile_pool(name="nps", bufs=1, space="PSUM"))
            sb_pool = ctx.enter_context(tc.tile_pool(name="gates", bufs=2))
            misc = ctx.enter_context(tc.tile_pool(name="misc", bufs=1))

            def _load(pool, dram, shape, tag, dt=f32):
                t_ = pool.tile(shape, dt, tag=tag)
                nc.sync.dma_start(t_[:], dram[:])
                return t_

            wihcT = _load(wpool, wihcT_d, [128, KE, 3 * H], "wihcT", f16)
            biasc = _load(wpool, biasc_d, [128, GC], "biasc")
            biascr = _load(wpool, biascr_d, [128, GC, HALF], "biascr")
            whh16 = _load(wpool, whh16_d, [128, KH, 3 * H], "whh16", f16)
            bhhnc_rep = _load(wpool, bhhnc_d, [128, KH, BPC], "bhhnc")
            wdiffT = _load(misc, wdiffT_d, [128, KH, 1], "wdiffT", f16)
            ncdiff = _load(misc, ncdiff_d, [1, BPC * t_len], "ncdiff")
            zs = misc.tile([1, 128], f16, tag="zs")
            zx = misc.tile([1, 4 * BPC], f16, tag="zx")
            nc.vector.memset(zs[:], 0.0)
            nc.vector.memset(zx[:], 0.0)

            # gi layout: [128, GC, BPC, t] (t innermost: contiguous proj copies)
            gi = gipool.tile([128, GC, BPC, t_len], f16, tag="gi")
            emit_prefix, groups = _proj_builder(
                nc, tc, misc, dma_pool, big_ps, embT_d, wihcT, biasc, biascr,
                gi, t_len, KE, act, alu, f16, f32, "proj")
            emit_prefix()

            hist = hpool.tile([128, KH, BPC, t_len], f16, tag="hist")
            rz_ps = rz_psp.tile([128, 4, BPC], f32, tag="rz")
            n_ps = n_psp.tile([128, KH, BPC], f32, tag="n")

            # prime has_written for the preload+accumulate banks
            nc.tensor.matmul(rz_ps[:], zs[:], zx[:], start=True, stop=True)
            nc.tensor.matmul(n_ps[:], zs[:], zx[0:1, 0: KH * BPC], start=True, stop=True)
            # initial preloads for t=0
            nc.vector.tensor_copy(rz_ps[:], gi[:, 0:4, :, 0])
            nc.scalar.activation(n_ps[:], bhhnc_rep[:], act.Identity)

            def _gh(w, rhs, chunks, last=False):
                """Accumulate Whh @ rhs into the gate banks (start=False)."""
                for c in chunks:
                    dst = rz_ps[:, c, :] if c < 4 else n_ps[:, c - 4, :]
                    for k in range(KH):
                        nc.tensor.matmul(
                            dst, w[:, k, c * 128: (c + 1) * 128], rhs[:, k, :],
                            start=False, stop=(last and c == chunks[-1] and k == KH - 1),
                            skip_group_check=True,
                        )

            def emit_step(t):
                # gh(t) was already accumulated by step t-1 (MM-split: the
                # Whh@nn part right after tanh, the Whh@d part after op8).
                # Chain: sigma_r -> x r -> +gi_n -> tanh -> (h_prev-nn) ->
                # x z -> MM_d; sigma_z, preloads, MM_nn, h-write off-chain.
                # h = (1-z)*nn + z*h_prev; both (1-z) and z*h_prev are
                # off-chain (z and h_prev are ready early), so the serial
                # chain after tanh is only the (1-z)*nn multiply before the
                # critical Whh@[(1-z)nn] matmul group.
                rz = sb_pool.tile([128, 4, BPC], f32, tag="rz")
                tmp = sb_pool.tile([128, KH, BPC], f32, tag="tmp")
                nc.scalar.activation(rz[:, 0:2, :], rz_ps[:, 0:2, :], act.Sigmoid)
                nc.scalar.activation(rz[:, 2:4, :], rz_ps[:, 2:4, :], act.Sigmoid)
                nc.vector.tensor_tensor(tmp[:], n_ps[:], rz[:, 0:KH, :], alu.mult)
                nc.vector.tensor_tensor(tmp[:], tmp[:], gi[:, 4:GC, :, t], alu.add)
                if t + 1 < t_len:
                    nc.vector.tensor_copy(rz_ps[:], gi[:, 0:4, :, t + 1])
                    nc.vector.tensor_copy(n_ps[:], bhhnc_rep[:])
                omz = sb_pool.tile([128, KH, BPC], f32, tag="omz")
                nc.vector.tensor_scalar(omz[:], rz[:, 2:4, :], -1.0, 1.0, alu.mult, alu.add)
                zh16 = sb_pool.tile([128, KH, BPC], f16, tag="zh")
                if t > 0:
                    nc.gpsimd.tensor_tensor(zh16[:], rz[:, 2:4, :], hist[:, :, :, t - 1], alu.mult)
                    if t + 1 < t_len:
                        _gh(whh16, zh16, (0, 1, 2, 3, 4, 5))
                nn16 = sb_pool.tile([128, KH, BPC], f16, tag="nn")
                nc.scalar.activation(nn16[:], tmp[:], act.Tanh)
                a16 = sb_pool.tile([128, KH, BPC], f16, tag="a16")
                nc.vector.tensor_tensor(a16[:], omz[:], nn16[:], alu.mult)
                if t + 1 < t_len:
                    _gh(whh16, a16, (0, 1, 2, 3, 4, 5), last=True)
                if t > 0:
                    nc.gpsimd.tensor_tensor(hist[:, :, :, t], a16[:], zh16[:], alu.add)
                else:
                    nc.gpsimd.tensor_copy(hist[:, :, :, t], a16[:])

            gidx = 0
            for t in range(t_len):
                emit_step(t)
                if t % 2 == 1 and gidx < len(groups):
                    groups[gidx]()
                    gidx += 1
            while gidx < len(groups):
                groups[gidx]()
                gidx += 1

            # ---- batched decisions: ks[b,t] = (h_t . wdiff > ncdiff) ----
            ks_sb = misc.tile([1, BPC * t_len], f32, tag="kssb")
            for b in range(BPC):
                dps = big_ps.tile([1, t_len], f32, tag="bps")
                for k in range(KH):
                    nc.tensor.matmul(
                        dps[:], wdiffT[:, k, :], hist[:, k, b, :],
                        start=(k == 0), stop=(k == KH - 1),
                    )
                nc.vector.tensor_tensor(
                    ks_sb[0:1, b * t_len: (b + 1) * t_len], dps[:],
                    ncdiff[0:1, b * t_len: (b + 1) * t_len], alu.is_gt,
                )
            nc.sync.dma_start(ks_d[:], ks_sb[:])

    return _split_excess_waits(nc)


def build_kernel2(t2, kf3, kf4, kf5):
    """GRU0/GRU1 + convs + pooling + final linear at dynamic length t2.

    Both layers use the k1-style low-latency step (PSUM preloads, split
    sigma, DVE-resident chain); no matmul split (two chains share the PE,
    so the per-wave PE budget matters more than each chain's MM segment).
    proj1 computed per-D-chunk from the o1 history into a gi1 buffer;
    conv max-pool windows (kf*) are compile-time constants."""
    import concourse.tile as tile
    from concourse import mybir

    _apply_tile_patch()
    nc = _mk_nc()
    f32 = mybir.dt.float32
    f16 = mybir.dt.float16
    act = mybir.ActivationFunctionType
    alu = mybir.AluOpType
    D = 32
    LAG = D + 8
    HALF = t2 // 2

    nembT_d = nc.dram_tensor("nembT", [KE, 128, BPC * t2], f16, kind="ExternalInput").ap()
    wih0T_d = nc.dram_tensor("wih0T", [128, KE, 3 * H], f16, kind="ExternalInput").ap()
    whh0T_d = nc.dram_tensor("whh0T", [128, KH, 3 * H], f16, kind="ExternalInput").ap()
    bias0_d = nc.dram_tensor("bias0", [128, GC], f32, kind="ExternalInput").ap()
    bias0r_d = nc.dram_tensor("bias0r", [128, GC, HALF], f32, kind="ExternalInput").ap()
    bhhn0_d = nc.dram_tensor("bhhn0", [128, KH, BPC], f32, kind="ExternalInput").ap()
    wih1T_d = nc.dram_tensor("wih1T", [128, KH, 3 * H], f16, kind="ExternalInput").ap()
    whh1T_d = nc.dram_tensor("whh1T", [128, KH, 3 * H], f16, kind="ExternalInput").ap()
    bias1_d = nc.dram_tensor("bias1", [128, GC], f32, kind="ExternalInput").ap()
    bias1r_d = nc.dram_tensor("bias1r", [128, GC, D], f32, kind="ExternalInput").ap()
    bhhn1_d = nc.dram_tensor("bhhn1", [128, KH, BPC], f32, kind="ExternalInput").ap()
    vt_d = nc.dram_tensor("vt", [1, BPC * t2], f16, kind="ExternalInput").ap()
    cw_d = nc.dram_tensor("cw", [128, 12, KH, NF], f16, kind="ExternalInput").ap()
    cb_d = nc.dram_tensor("cb", [NF, 3], f32, kind="ExternalInput").ap()
    woutT_d = nc.dram_tensor("woutT", [NF, 3], f32, kind="ExternalInput").ap()
    bout_d = nc.dram_tensor("bout", [1, 1], f32, kind="ExternalInput").ap()
    out_d = nc.dram_tensor("out", [1, BPC], f32, kind="ExternalOutput").ap()

    FS = (3, 4, 5)
    KFS = (kf3, kf4, kf5)

    with tile.TileContext(nc) as tc:
        from contextlib import ExitStack

        with ExitStack() as ctx:
            wpool = ctx.enter_context(tc.tile_pool(name="weights", bufs=1))
            gipool = ctx.enter_context(tc.tile_pool(name="gi", bufs=1))
            opool = ctx.enter_context(tc.tile_pool(name="obuf", bufs=1))
            dma_pool = ctx.enter_context(tc.tile_pool(name="dma", bufs=2))
            big_ps = ctx.enter_context(tc.tile_pool(name="bigps", bufs=2, space="PSUM"))
            rz0_psp = ctx.enter_context(tc.tile_pool(name="rz0ps", bufs=1, space="PSUM"))
            n0_psp = ctx.enter_context(tc.tile_pool(name="n0ps", bufs=1, space="PSUM"))
            rz1_psp = ctx.enter_context(tc.tile_pool(name="rz1ps", bufs=1, space="PSUM"))
            n1_psp = ctx.enter_context(tc.tile_pool(name="n1ps", bufs=1, space="PSUM"))
            sb_pool = ctx.enter_context(tc.tile_pool(name="gates", bufs=2))
            misc = ctx.enter_context(tc.tile_pool(name="misc", bufs=1))

            def _load(pool, dram, shape, tag, dt=f32):
                t_ = pool.tile(shape, dt, tag=tag)
                nc.sync.dma_start(t_[:], dram[:])
                return t_

            wih0T = _load(wpool, wih0T_d, [128, KE, 3 * H], "wih0", f16)
            whh0T = _load(wpool, whh0T_d, [128, KH, 3 * H], "whh0", f16)
            bias0 = _load(wpool, bias0_d, [128, GC], "bias0")
            bias0r = _load(wpool, bias0r_d, [128, GC, HALF], "bias0r")
            bhhn0 = _load(wpool, bhhn0_d, [128, KH, BPC], "bhhn0")
            wih1T = _load(wpool, wih1T_d, [128, KH, 3 * H], "wih1", f16)
            whh1T = _load(wpool, whh1T_d, [128, KH, 3 * H], "whh1", f16)
            bias1 = _load(wpool, bias1_d, [128, GC], "bias1")
            bias1r = _load(wpool, bias1r_d, [128, GC, D], "bias1r")
            bhhn1 = _load(wpool, bhhn1_d, [128, KH, BPC], "bhhn1")
            cw = _load(wpool, cw_d, [128, 12, KH, NF], "cw", f16)
            cb = _load(misc, cb_d, [NF, 3], "cb")
            woutT = _load(misc, woutT_d, [NF, 3], "woutT")
            bout = _load(misc, bout_d, [1, 1], "bout")
            vt = _load(misc, vt_d, [1, BPC * t2], "vt", f16)
            zs = misc.tile([1, 128], f16, tag="zs")
            zx = misc.tile([1, 4 * BPC], f16, tag="zx")
            nc.vector.memset(zs[:], 0.0)
            nc.vector.memset(zx[:], 0.0)

            gi0 = gipool.tile([128, GC, BPC, t2], f16, tag="gi0")
            gi1 = gipool.tile([128, GC, BPC, t2], f16, tag="gi1")
            o1 = opool.tile([128, KH, BPC, t2], f16, tag="o1")
            o2 = opool.tile([128, KH, BPC, t2], f16, tag="o2")

            emit_prefix, groups = _proj_builder(
                nc, tc, misc, dma_pool, big_ps, nembT_d, wih0T, bias0, bias0r,
                gi0, t2, KE, act, alu, f16, f32, "proj0")
            emit_prefix()

            rz0 = rz0_psp.tile([128, 4, BPC], f32, tag="rz0")
            n0 = n0_psp.tile([128, KH, BPC], f32, tag="n0")
            rz1 = rz1_psp.tile([128, 4, BPC], f32, tag="rz1")
            n1 = n1_psp.tile([128, KH, BPC], f32, tag="n1")

            for ps_t in (rz0, rz1):
                nc.tensor.matmul(ps_t[:], zs[:], zx[:], start=True, stop=True)
            for ps_t in (n0, n1):
                nc.tensor.matmul(ps_t[:], zs[:], zx[0:1, 0: KH * BPC], start=True, stop=True)
            # initial preloads
            nc.vector.tensor_copy(rz0[:], gi0[:, 0:4, :, 0])
            nc.scalar.activation(n0[:], bhhn0[:], act.Identity)
            nc.scalar.activation(n1[:], bhhn1[:], act.Identity)

            def emit_step(t, gi, whh, bhhn, hist, prev, rz_ps, n_ps, sfx, tl):
                """One GRU layer step, matmul-split form: gh(t+1) is
                accumulated as Whh@nn16 (after tanh) + Whh@d16 (after the
                z-mult), so the h-write and the full burst leave the chain."""
                def gh(rhs, last):
                    for c in (0, 1, 2, 3, 4, 5):
                        dst = rz_ps[:, c, :] if c < 4 else n_ps[:, c - 4, :]
                        for k in range(KH):
                            nc.tensor.matmul(
                                dst, whh[:, k, c * 128: (c + 1) * 128],
                                rhs[:, k, :],
                                start=False, stop=(last and k == KH - 1),
                                skip_group_check=True,
                            )
                rz = sb_pool.tile([128, 4, BPC], f32, tag="rz" + sfx)
                tmp = sb_pool.tile([128, KH, BPC], f32, tag="tmp" + sfx)
                nc.scalar.activation(rz[:], rz_ps[:], act.Sigmoid)
                nc.vector.tensor_tensor(tmp[:], n_ps[:], rz[:, 0:KH, :], alu.mult)
                if t + 1 < tl:
                    nc.vector.tensor_copy(rz_ps[:], gi[:, 0:4, :, t + 1])
                nc.gpsimd.tensor_tensor(tmp[:], tmp[:], gi[:, 4:GC, :, t], alu.add)
                nn16 = sb_pool.tile([128, KH, BPC], f16, tag="nn" + sfx)
                nc.scalar.activation(nn16[:], tmp[:], act.Tanh)
                if t + 1 < tl:
                    nc.scalar.activation(n_ps[:], bhhn[:], act.Identity)
                    gh(nn16, last=False)
                d = sb_pool.tile([128, KH, BPC], f32, tag="dd" + sfx)
                if t > 0:
                    nc.vector.tensor_tensor(d[:], hist[:, :, :, t - 1], nn16[:], alu.subtract)
                else:
                    nc.vector.tensor_scalar(d[:], nn16[:], -1.0, None, alu.mult)
                d16 = sb_pool.tile([128, KH, BPC], f16, tag="d6" + sfx)
                nc.vector.tensor_tensor(d16[:], rz[:, 2:4, :], d[:], alu.mult)
                if t + 1 < tl:
                    gh(d16, last=True)
                nc.gpsimd.tensor_tensor(hist[:, :, :, t], nn16[:], d16[:], alu.add)

            def emit_proj1_chunk(ci):
                t0, t1 = ci * D, (ci + 1) * D
                for b in range(BPC):
                    for c in range(GC):
                        ps = big_ps.tile([128, D], f32, tag="bps")
                        for k in range(KH):
                            nc.tensor.matmul(
                                ps[:], wih1T[:, k, c * 128: (c + 1) * 128],
                                o1[:, k, b, t0:t1],
                                start=(k == 0), stop=(k == KH - 1),
                            )
                        dst = gi1[:, c, b, t0:t1]
                        if (b * GC + c) % 2 == 0:
                            nc.scalar.activation(
                                dst, ps[:], act.Identity, bias=bias1[:, c: c + 1])
                        else:
                            nc.vector.tensor_tensor(dst, ps[:], bias1r[:, c, :], alu.add)

            l1_started = [False]
            gidx = 0
            for w in range(t2 + LAG):
                if w < t2:
                    emit_step(w, gi0, whh0T, bhhn0, o1, o1, rz0, n0, "0", t2)
                    if w % 2 == 1 and gidx < len(groups):
                        groups[gidx]()
                        gidx += 1
                if w >= LAG:
                    t = w - LAG
                    if not l1_started[0]:
                        nc.vector.tensor_copy(rz1[:], gi1[:, 0:4, :, 0])
                        l1_started[0] = True
                    emit_step(t, gi1, whh1T, bhhn1, o2, o2, rz1, n1, "1", t2)
                if w < t2 and w % D == D - 1:
                    emit_proj1_chunk(w // D)
            while gidx < len(groups):
                groups[gidx]()
                gidx += 1

            # ---- zero o2 past new_lens: o2 *= vt ----
            # partition-broadcast vt via a K=1 ones-matmul (PE outer product)
            ones_sb = misc.tile([1, 128], f16, tag="ones")
            nc.vector.memset(ones_sb[:], 1.0)
            for b in range(BPC):
                vtb = big_ps.tile([128, t2], f32, tag="bps")
                nc.tensor.matmul(
                    vtb[:], ones_sb[:], vt[0:1, b * t2: (b + 1) * t2],
                    start=True, stop=True,
                )
                for k in range(KH):
                    nc.vector.tensor_tensor(
                        o2[:, k, b, :], o2[:, k, b, :], vtb[:], alu.mult
                    )

            # ---- convs + relu + max-pool over compile-time window ----
            pooled = misc.tile([NF, 3, BPC], f32, tag="pooled")
            for b in range(BPC):
                for fi, fs in enumerate(FS):
                    nw = t2 - fs + 1
                    kf = KFS[fi]
                    ps = big_ps.tile([NF, t2], f32, tag="bps")
                    m0 = sum(FS[:fi])  # flat (fs,dt) base index
                    first = True
                    for dt_ in range(fs):
                        for k in range(KH):
                            nc.tensor.matmul(
                                ps[:, :nw],
                                cw[:, m0 + dt_, k, :],
                                o2[:, k, b, dt_: dt_ + nw],
                                start=first,
                                stop=(dt_ == fs - 1 and k == KH - 1),
                            )
                            first = False
                    crelu = sb_pool.tile([NF, t2], f32, tag="crelu")
                    nc.scalar.activation(
                        crelu[:, :kf], ps[:, :kf], act.Relu, bias=cb[:, fi: fi + 1]
                    )
                    nc.vector.tensor_reduce(
                        pooled[:, fi, b: b + 1], crelu[:, :kf],
                        mybir.AxisListType.X, alu.max,
                    )

            # ---- final linear ----
            fps = big_ps.tile([1, BPC], f32, tag="bps")
            for fi in range(3):
                nc.tensor.matmul(
                    fps[:], woutT[:, fi: fi + 1], pooled[:, fi, :],
                    start=(fi == 0), stop=(fi == 2),
                )
            out_sb = misc.tile([1, BPC], f32, tag="outsb")
            nc.scalar.activation(out_sb[:], fps[:], act.Identity, bias=bout[0:1, 0:1])
            nc.sync.dma_start(out_d[:], out_sb[:])

    return _split_excess_waits(nc)


# ------------------------------------------------------------- host orchestration
def _host_pack_k1(inputs, gumbel, t_len=T):
    f16 = _np_f16()
    emb = np.asarray(inputs["embedded"], np.float32)
    mask = np.asarray(inputs["mask"])
    lens = mask.sum(1)
    maxlen = int(lens.max())

    wihcT, whhT, biasc, bhhnc = _pack_gru_weights(
        inputs["Wih_c"], inputs["Whh_c"], inputs["bih_c"], inputs["bhh_c"])
    wdiff = (inputs["Wsel"][1] - inputs["Wsel"][0]).astype(np.float32)
    wdiffT = np.ascontiguousarray(wdiff.reshape(KH, 128).T[:, :, None])
    bdiff = float(inputs["bsel"][1] - inputs["bsel"][0])

    # ncdiff[b, t]: k_t = (h.wdiff > ncdiff); forced off when t >= maxlen-1
    ncdiff = np.full((B, t_len), 1.0e30, np.float32)
    upto = min(maxlen - 1, t_len)
    for t in range(1, upto):
        ncdiff[:, t] = -(bdiff + gumbel[t - 1, :, 1] - gumbel[t - 1, :, 0])

    biascr = np.ascontiguousarray(
        np.broadcast_to(biasc[:, :, None], (128, GC, t_len // 2))).astype(np.float32)

    in_maps = []
    for c in range(NCORES):
        rows = slice(c * BPC, (c + 1) * BPC)
        in_maps.append({
            "embT": _pack_embT(emb[rows, :t_len], t_len).astype(f16),
            "wihcT": wihcT.astype(f16),
            "biasc": biasc,
            "biascr": biascr,
            "whh16": whhT.astype(f16),
            "bhhnc": bhhnc,
            "wdiffT": wdiffT.astype(f16),
            "ncdiff": np.ascontiguousarray(
                ncdiff[rows].reshape(1, BPC * t_len)),
        })
    return in_maps, lens, maxlen


def _host_compact(inputs, ks_full, lens, maxlen, t_len=T):
    """ks_full: [B, t_len] decision bits (row t=0 ignored; selected[:,0]=1)."""
    emb = np.asarray(inputs["embedded"], np.float32)
    selected = np.zeros((B, t_len), np.int64)
    selected[:, 0] = 1
    selected[:, 1:] = ks_full[:, 1:]
    pos = np.arange(t_len)
    sel_valid = np.where(pos[None, :] < (lens - 1)[:, None], selected, 0)
    new_mask = np.where(pos[None, :] == (lens - 1)[:, None], 1, sel_valid)
    new_lens = new_mask.sum(1)
    Ldyn = max(int(new_lens.max()), 7)

    t2 = max(-(-Ldyn // 64) * 64, 64)
    new_emb = np.zeros((B, t2, E), np.float32)
    for b in range(B):
        idx = np.nonzero(new_mask[b])[0]
        new_emb[b, : len(idx)] = emb[b, idx]
    return new_emb, new_lens, Ldyn, t2


def _host_pack_k2(inputs, new_emb, new_lens, Ldyn, t2):
    f16 = _np_f16()
    wih0T, whh0T, bias0, bhhn0 = _pack_gru_weights(
        inputs["Wih0"], inputs["Whh0"], inputs["bih0"], inputs["bhh0"])
    wih1T, whh1T, bias1, bhhn1 = _pack_gru_weights(
        inputs["Wih1"], inputs["Whh1"], inputs["bih1"], inputs["bhh1"])

    FS = (3, 4, 5)
    cw = np.zeros((128, 12, KH, NF), np.float32)
    cb = np.zeros((NF, 3), np.float32)
    m = 0
    for fi, fs in enumerate(FS):
        w = np.asarray(inputs[f"conv_w{fs}"], np.float32)  # [NF,1,fs,H]
        cb[:, fi] = np.asarray(inputs[f"conv_b{fs}"], np.float32)
        for dt_ in range(fs):
            wt = w[:, 0, dt_, :].T  # [H, NF]
            cw[:, m, :, :] = wt.reshape(KH, 128, NF).transpose(1, 0, 2)
            m += 1

    woutT = np.ascontiguousarray(
        np.asarray(inputs["Wout"], np.float32)[0].reshape(3, NF).T)
    bout = np.asarray(inputs["bout"], np.float32).reshape(1, 1)

    vt_full = (np.arange(t2)[None, :] < new_lens[:, None]).astype(np.float32)

    bias0r = np.ascontiguousarray(
        np.broadcast_to(bias0[:, :, None], (128, GC, t2 // 2))).astype(np.float32)
    bias1r = np.ascontiguousarray(
        np.broadcast_to(bias1[:, :, None], (128, GC, 32))).astype(np.float32)

    in_maps = []
    for c in range(NCORES):
        rows = slice(c * BPC, (c + 1) * BPC)
        in_maps.append({
            "nembT": _pack_embT(new_emb[rows], t2).astype(f16),
            "wih0T": wih0T.astype(f16), "whh0T": whh0T.astype(f16),
            "bias0": bias0, "bias0r": bias0r, "bhhn0": bhhn0,
            "wih1T": wih1T.astype(f16), "whh1T": whh1T.astype(f16),
            "bias1": bias1, "bias1r": bias1r, "bhhn1": bhhn1,
            "vt": np.ascontiguousarray(
                vt_full[rows].reshape(1, BPC * t2)).astype(f16),
            "cw": cw.astype(f16), "cb": cb,
            "woutT": woutT, "bout": bout,
        })
    return in_maps


_NC_CACHE = {}


def _get_nc1(t_len=T):
    key = (1, t_len)
    if key not in _NC_CACHE:
        _NC_CACHE[key] = build_kernel1(t_len)
    return _NC_CACHE[key]


def _get_nc2(t2, kfs):
    key = (2, t2, kfs)
    if key not in _NC_CACHE:
        _NC_CACHE[key] = build_kernel2(t2, *kfs)
    return _NC_CACHE[key]


TRACE = False  # set True (with an NTFF hook registered) to collect exec times
LAST_STATS = {}


def kernel(**inputs):
    from concourse import bass_utils

    gumbel = _gumbel_cpu()
    core_ids = list(range(NCORES))

    in_maps1, lens, maxlen = _host_pack_k1(inputs, gumbel)
    nc1 = _get_nc1()
    res1 = bass_utils.run_bass_kernel_spmd(nc1, in_maps1, core_ids, trace=TRACE)
    ks_full = np.concatenate(
        [res1.results[c]["ks"].reshape(BPC, T) for c in range(NCORES)], axis=0)

    new_emb, new_lens, Ldyn, t2 = _host_compact(inputs, ks_full, lens, maxlen)
    kfs = tuple(min(Ldyn - fs + 1, t2 - fs + 1) for fs in (3, 4, 5))
    in_maps2 = _host_pack_k2(inputs, new_emb, new_lens, Ldyn, t2)
    nc2 = _get_nc2(t2, kfs)
    res2 = bass_utils.run_bass_kernel_spmd(nc2, in_maps2, core_ids, trace=TRACE)
    out = np.concatenate([res2.results[c]["out"][0] for c in range(NCORES)], axis=0)
    LAST_STATS["k1_ns"] = res1.exec_time_ns
    LAST_STATS["k2_ns"] = res2.exec_time_ns
    LAST_STATS["ks"] = ks_full
    LAST_STATS["new_lens"] = new_lens
    return out.astype(np.float32)
